# revision 6
# baseline (speedup 1.0000x reference)
"""Trainium2 Bass kernel for CodeAttention (B=4, S=2048, E=768, H=12).

Sharding: 8 cores = 4 batches x 2 head-groups (6 heads each).
Each core computes a partial projection output for its batch; the host
sums the two partials per batch and adds the (host-folded) bias row.

v2: fp16 datapath. The padding mask is folded multiplicatively into the
V store (masked keys get v=0 AND ones-column=0), so exp needs no per-
key-chunk bias and one activation instruction covers 2 key chunks x 2
heads (2048 cols) — keeping the scalar engine off the critical path.
"""

import sys

if "/opt/trn_rl_repo" not in sys.path:
    sys.path.insert(0, "/opt/trn_rl_repo")

import numpy as np

import concourse.bass as bass  # noqa: F401  (engine types referenced via nc)
import concourse.mybir as mybir
import concourse.tile as tile
from concourse import bacc
from concourse.alu_op_type import AluOpType
from concourse.bass_utils import run_bass_kernel_spmd
from concourse.masks import make_identity

F32 = mybir.dt.float32
F32R = mybir.dt.float32r
FP16 = mybir.dt.float16
Act = mybir.ActivationFunctionType

B, S, E, H, D = 4, 2048, 768, 12, 64
HC = 6                    # heads per core
QKC = HC * D * 2          # qk columns per core = 768
VC = HC * D               # v columns per core = 384
KCH = E // 128            # contraction chunks over E = 6
NKC = S // 128            # key chunks = 16
NQB = S // 512            # q blocks of 512 = 4
NSB = S // 512            # s blocks of 512 = 4
VW = D + 1                # v width incl. mask column = 65


def build_program():
    nc = bacc.Bacc("TRN2", target_bir_lowering=False, debug=False, num_devices=8)

    x_d = nc.dram_tensor("x", [S, E], FP16, kind="ExternalInput")
    wqk_d = nc.dram_tensor("wqk", [KCH, 128, QKC], FP16, kind="ExternalInput")
    wv_d = nc.dram_tensor("wv", [KCH, 128, VC], FP16, kind="ExternalInput")
    wp_d = nc.dram_tensor("wp", [VC // 128, 128, E], FP16, kind="ExternalInput")
    bqk_d = nc.dram_tensor("bqk", [QKC], F32, kind="ExternalInput")
    mb_d = nc.dram_tensor("mb", [S], F32, kind="ExternalInput")
    mrep_d = nc.dram_tensor("mrep", [NKC * HC * 128], FP16, kind="ExternalInput")
    y_d = nc.dram_tensor("y", [S, E], F32, kind="ExternalOutput")

    with tile.TileContext(nc) as tc:
        _emit(nc, tc, x_d, wqk_d, wv_d, wp_d, bqk_d, mb_d, mrep_d, y_d)
    nc.compile()
    return nc


def _emit(nc, tc, x_d, wqk_d, wv_d, wp_d, bqk_d, mb_d, mrep_d, y_d):
    ctx_pools = []

    def pool(name, bufs, space="SBUF"):
        p = tc.tile_pool(name=name, bufs=bufs, space=space)
        ctx_pools.append(p)
        return p.__enter__()

    consts = pool("consts", 1)
    store = pool("store", 1)

    ident = consts.tile([128, 128], FP16)
    make_identity(nc, ident[:])

    # weights go over the SWDGE (gpsimd) queue so the x-chunk loads on the
    # sync HWDGE queue aren't serialized behind the weight traffic.
    wqk = consts.tile([128, KCH, QKC], FP16)
    wv = consts.tile([128, KCH, VC], FP16)
    wp = consts.tile([128, VC // 128, E], FP16)
    for k in range(KCH):
        nc.gpsimd.dma_start(wv[:, k, :], wv_d.ap()[k])
    for k in range(KCH):
        nc.gpsimd.dma_start(wqk[:, k, :], wqk_d.ap()[k])
    for t in range(VC // 128):
        nc.gpsimd.dma_start(wp[:, t, :], wp_d.ap()[t])

    bqk = consts.tile([128, QKC // 128], F32)
    nc.scalar.dma_start(bqk[:], bqk_d.ap().rearrange("(c p) -> p c", p=128))
    mb = consts.tile([128, NKC], F32)
    nc.scalar.dma_start(mb[:], mb_d.ap().rearrange("(c p) -> p c", p=128))
    # mask replicated per head: mrep[p, kc, h] = mask[kc*128 + p]
    mrep = consts.tile([128, NKC, HC], FP16)
    nc.scalar.dma_start(
        mrep[:], mrep_d.ap().rearrange("(c h p) -> p c h", p=128, h=HC)
    )

    # qkT store, one tile per s-block so attention deps are per-block:
    # tile m of 6 holds W-columns m*128..; q cols 0..383, k cols 384..767.
    qkT = [
        store.tile([128, QKC // 128, 512], FP16, name=f"qkT{sb}")
        for sb in range(NSB)
    ]
    # v store: per s-block [s-chunk, head, 65] with the mask value (0/1) in
    # column 64 — multiplicative padding mask (masked keys contribute 0 to
    # both the numerator and the softmax denominator).
    vst = [
        store.tile([128, 4, HC, VW], FP16, name=f"vst{sb}") for sb in range(NSB)
    ]
    for sb in range(NSB):
        nc.vector.tensor_copy(
            vst[sb][:, :, :, D : D + 1],
            mrep[:, sb * 4 : (sb + 1) * 4, :].rearrange(
                "p a (b one) -> p a b one", b=HC, one=1
            ),
        )
    # attn output (transposed): tile t rows = head dims 2t,2t+1.
    att = store.tile([128, VC // 128, S], FP16)

    # ---- Phase 1: QKV projections ----
    with (
        tc.tile_pool(name="xs", bufs=3) as xs_p,
        tc.tile_pool(name="xt", bufs=3) as xt_p,
        tc.tile_pool(name="tp", bufs=3, space="PSUM") as tp_p,
        tc.tile_pool(name="va", bufs=2, space="PSUM") as va_p,
        tc.tile_pool(name="qk", bufs=3, space="PSUM") as qk_p,
    ):
        _emit_qkv(
            nc, x_d, ident, wqk, wv, bqk, mb, qkT, vst, xs_p, xt_p, tp_p, va_p,
            qk_p,
        )

    # ---- Phase 2: attention + projection ----
    st_p = pool("st", 2, space="PSUM")       # [128,2048] fp16 = 2 banks each
    pv_p = pool("pv", 4, space="PSUM")       # pvs + ya rotation
    pt_p = pool("pt", 3)
    rs_p = pool("rs", 2)
    bc_p = pool("bc", 2)
    ys_p = pool("ys", 2)

    for qb in range(NQB):
        qs = slice(qb * 512, (qb + 1) * 512)
        deferred_norm = None
        for hp in range(HC // 2):
            pvs = [
                pv_p.tile([128, 512], F32, tag="pv", name=f"pv{qb}_{hp}_{i}")
                for i in range(2)
            ]
            for kc in range(NKC):
                # both heads of the pair share one 2-bank score tile so one
                # exp instruction (free dim 1024) covers both; matmul PSUM
                # output must be f32 on TRN2 so this is the merge limit.
                st = st_p.tile([128, 1024], F32, tag="st")
                for sub in range(2):
                    kb, ko = kc // 4, kc % 4
                    r0 = sub * 64
                    nc.tensor.matmul(
                        st[:, sub * 512 : (sub + 1) * 512],
                        qkT[kb][r0 : r0 + 64, 3 + hp, ko * 128 : (ko + 1) * 128],
                        qkT[qb][r0 : r0 + 64, hp, :],
                        start=True, stop=True,
                    )
                pt = pt_p.tile([128, 1024], FP16, tag="pt")
                nc.scalar.activation(pt[:], st[:], Act.Exp, scale=0.125)
                for sub in range(2):
                    h = hp * 2 + sub
                    nc.tensor.matmul(
                        pvs[sub][0:VW, :], vst[kc // 4][:, kc % 4, h, :],
                        pt[:, sub * 512 : (sub + 1) * 512],
                        start=(kc == 0), stop=(kc == NKC - 1),
                    )
                if kc == 2 and deferred_norm is not None:
                    deferred_norm()
                    deferred_norm = None
            def norm_pair(pvs=pvs, hp=hp, qs=qs):
                for sub in range(2):
                    # reciprocal of the softmax denominator row, then rank-1
                    # broadcast to 64 partitions on the (idle) gpsimd engine.
                    rse = rs_p.tile([1, 512], F32R, tag="rs", name="rse")
                    with nc.allow_low_precision(reason="f32r is full width"):
                        nc.vector.reciprocal(rse[:], pvs[sub][D : D + 1, :])
                    bct = bc_p.tile([D, 512], F32R, tag="bc", name="bct")
                    nc.gpsimd.partition_broadcast(bct[:], rse[:])
                    nc.vector.tensor_tensor(
                        att[sub * 64 : sub * 64 + 64, hp, qs],
                        pvs[sub][0:D, :], bct[:], op=AluOpType.mult,
                    )
            deferred_norm = norm_pair
        if deferred_norm is not None:
            deferred_norm()
            deferred_norm = None
        # projection for this q-block
        for sc in range(4):
            sg = qb * 4 + sc
            ys = ys_p.tile([128, E], F32, tag="ys")
            for n0, nw in ((0, 512), (512, 256)):
                ya = pv_p.tile([128, 512], F32, tag="pv", name="ya")
                for t in range(VC // 128):
                    nc.tensor.matmul(
                        ya[:, :nw],
                        att[:, t, sg * 128 : (sg + 1) * 128],
                        wp[:, t, n0 : n0 + nw],
                        start=(t == 0), stop=(t == VC // 128 - 1),
                    )
                nc.vector.tensor_copy(ys[:, n0 : n0 + nw], ya[:, :nw])
            nc.sync.dma_start(y_d.ap()[sg * 128 : (sg + 1) * 128, :], ys[:])

    for p in reversed(ctx_pools):
        p.__exit__(None, None, None)


def _emit_qkv(nc, x_d, ident, wqk, wv, bqk, mb, qkT, vst, xs_p, xt_p, tp_p, va_p, qk_p):
    for sb in range(NSB):
        xt = xt_p.tile([128, KCH, 512], FP16)
        for sc in range(4):
            sg = sb * 4 + sc
            xs = xs_p.tile([128, E], FP16)
            nc.sync.dma_start(xs[:], x_d.ap()[sg * 128 : (sg + 1) * 128, :])
            # 6 fp16 transposes share one PSUM bank (2KB = 1024 fp16) as one
            # accumulation group (disjoint columns, per-element has_written).
            tp = tp_p.tile([128, 1024], FP16, tag="tp")
            for k in range(KCH):
                nc.tensor.matmul(
                    tp[:, k * 128 : (k + 1) * 128],
                    xs[:, k * 128 : (k + 1) * 128], ident[:],
                    is_transpose=True,
                    start=(k == 0), stop=(k == KCH - 1),
                )
            nc.vector.tensor_copy(
                xt[:, :, sc * 128 : (sc + 1) * 128],
                tp[:, : KCH * 128].rearrange("p (k f) -> p k f", k=KCH),
            )
            va = va_p.tile([128, VC], F32)
            for k in range(KCH):
                nc.tensor.matmul(
                    va[:], xt[:, k, sc * 128 : (sc + 1) * 128], wv[:, k, :],
                    start=(k == 0), stop=(k == KCH - 1),
                )
            # multiplicative padding mask folded into the V store (the mask
            # value for key row p is a per-partition scalar here).
            nc.vector.tensor_scalar_mul(
                vst[sb][:, sc, :, 0:D],
                va[:].rearrange("p (h d) -> p h d", h=HC),
                mb[:, sg : sg + 1],
            )
        for m in range(QKC // 128):
            qk = qk_p.tile([128, 512], F32)
            for k in range(KCH):
                nc.tensor.matmul(
                    qk[:], wqk[:, k, m * 128 : (m + 1) * 128], xt[:, k, :],
                    start=(k == 0), stop=(k == KCH - 1),
                )
            nc.vector.tensor_scalar_add(
                qkT[sb][:, m, :], qk[:], bqk[:, m : m + 1]
            )


def make_core_inputs(x, mask, Wqkv, bqkv):
    """Slice full inputs into 8 per-core input maps."""
    x = np.asarray(x, dtype=np.float32)
    mask = np.asarray(mask)
    Wqkv = np.asarray(Wqkv, dtype=np.float32)
    bqkv = np.asarray(bqkv, dtype=np.float32)
    in_maps = []
    for c in range(8):
        b = c // 2
        h0 = (c % 2) * HC
        wq = Wqkv[:, h0 * D : (h0 + HC) * D]
        wk = Wqkv[:, E + h0 * D : E + (h0 + HC) * D]
        wqk = np.concatenate([wq, wk], axis=1).reshape(KCH, 128, QKC)
        wv = Wqkv[:, 2 * E + h0 * D : 2 * E + (h0 + HC) * D].reshape(KCH, 128, VC)
        bqk = np.concatenate(
            [bqkv[h0 * D : (h0 + HC) * D], bqkv[E + h0 * D : E + (h0 + HC) * D]]
        )
        mv = (mask[b, 0, 0, :] != 0).astype(np.float32)  # 1.0 keep, 0.0 drop
        mrep = np.repeat(
            mv.reshape(NKC, 1, 128), HC, axis=1
        ).astype(np.float16).ravel()
        in_maps.append(
            {
                "x": np.ascontiguousarray(x[b].astype(np.float16)),
                "wqk": np.ascontiguousarray(wqk.astype(np.float16)),
                "wv": np.ascontiguousarray(wv.astype(np.float16)),
                "wp": None,  # filled below (needs Wproj)
                "bqk": np.ascontiguousarray(bqk.astype(np.float32)),
                "mb": np.ascontiguousarray(mv),
                "mrep": np.ascontiguousarray(mrep),
            }
        )
    return in_maps


def run(x, mask, Wqkv, bqkv, Wproj, bproj, trace=False, trace_cores=None):
    Wproj = np.asarray(Wproj, dtype=np.float32)
    bproj = np.asarray(bproj, dtype=np.float32)
    bqkv_np = np.asarray(bqkv, dtype=np.float32)
    in_maps = make_core_inputs(x, mask, Wqkv, bqkv_np)
    for c in range(8):
        h0 = (c % 2) * HC
        wp = Wproj[h0 * D : (h0 + HC) * D, :].reshape(VC // 128, 128, E)
        in_maps[c]["wp"] = np.ascontiguousarray(wp.astype(np.float16))

    nc = build_program()
    try:
        res = run_bass_kernel_spmd(
            nc, in_maps, core_ids=list(range(8)), trace=trace,
            trace_cores=trace_cores,
        )
    except Exception:
        # transient device wedge (e.g. NRT_EXEC_UNIT_UNRECOVERABLE) —
        # one retry is usually enough
        res = run_bass_kernel_spmd(
            nc, in_maps, core_ids=list(range(8)), trace=trace,
            trace_cores=trace_cores,
        )
    parts = [res.results[c]["y"] for c in range(8)]

    # host-folded bias: v-bias passes through softmax (weights sum to 1),
    # so y += bv @ Wproj + bproj, applied once per batch row.
    bv = bqkv_np[2 * E : 3 * E]
    bias_row = bv @ Wproj + bproj
    y = np.stack(
        [parts[2 * b] + parts[2 * b + 1] + bias_row for b in range(B)]
    ).astype(np.float32)
    return y, res


def kernel(x, mask, Wqkv, bqkv, Wproj, bproj):
    y, _ = run(x, mask, Wqkv, bqkv, Wproj, bproj, trace=False)
    return y


# revision 12
# speedup vs baseline: 1.1061x; 1.1061x over previous
"""Trainium2 Bass kernel for CodeAttention (B=4, S=2048, E=768, H=12).

Sharding: 8 cores = 4 batches x 2 head-groups (6 heads each).
Each core computes a partial projection output for its batch; the host
sums the two partials per batch and adds the (host-folded) bias row.

v3: fp16 datapath, single fused pipeline.
- The padding mask is folded multiplicatively into the V store (masked
  keys get v=0 AND ones-column=0), so exp needs no per-key-chunk bias.
- The attention kc loop is software-pipelined: scores(i) are emitted
  before pv(i-1) so the scalar engine's exp stream never waits on PE.
- QKV projection / output projection work is emitted as "filler units"
  inside the ACT-paced attention windows, keeping PE busy during exp.
"""

import sys

if "/opt/trn_rl_repo" not in sys.path:
    sys.path.insert(0, "/opt/trn_rl_repo")

import numpy as np

import concourse.bass as bass  # noqa: F401  (engine types referenced via nc)
import concourse.mybir as mybir
import concourse.tile as tile
from concourse import bacc
from concourse.alu_op_type import AluOpType
from concourse.bass_utils import run_bass_kernel_spmd
from concourse.masks import make_identity

F32 = mybir.dt.float32
F32R = mybir.dt.float32r
FP16 = mybir.dt.float16
Act = mybir.ActivationFunctionType

B, S, E, H, D = 4, 2048, 768, 12, 64
HC = 6                    # heads per core
QKC = HC * D * 2          # qk columns per core = 768
VC = HC * D               # v columns per core = 384
KCH = E // 128            # contraction chunks over E = 6
NKC = S // 128            # key chunks = 16
NQB = S // 512            # q blocks of 512 = 4
NSB = S // 512            # s blocks of 512 = 4
VW = D + 1                # v width incl. mask column = 65
NIT = NQB * (HC // 2) * NKC  # flattened attention iterations = 192


def build_program():
    nc = bacc.Bacc("TRN2", target_bir_lowering=False, debug=False, num_devices=8)

    x_d = nc.dram_tensor("x", [S, E], FP16, kind="ExternalInput")
    wqk_d = nc.dram_tensor("wqk", [KCH, 128, QKC], FP16, kind="ExternalInput")
    wv_d = nc.dram_tensor("wv", [KCH, 128, VC], FP16, kind="ExternalInput")
    wp_d = nc.dram_tensor("wp", [VC // 128, 128, E], FP16, kind="ExternalInput")
    bqk_d = nc.dram_tensor("bqk", [QKC], F32, kind="ExternalInput")
    mb_d = nc.dram_tensor("mb", [S], F32, kind="ExternalInput")
    mrep_d = nc.dram_tensor("mrep", [NKC * HC * 128], FP16, kind="ExternalInput")
    y_d = nc.dram_tensor("y", [S, E], F32, kind="ExternalOutput")

    with tile.TileContext(nc) as tc:
        _emit(nc, tc, x_d, wqk_d, wv_d, wp_d, bqk_d, mb_d, mrep_d, y_d)
    nc.compile()
    return nc


def _build_schedule():
    """fillers[i] = list of units to emit inside attention iteration i.

    Units: ("v", sb, sc) v-projection for s-chunk, ("qk", sb, m) qk
    projection m-chunk, ("norm", qb, hp) softmax normalization,
    ("proj", qb, sc) output projection chunk.  Lead-2 scheduling: a unit
    lands >=2 iterations before its first consumer so DVE evacuation of
    the unit's PSUM tile is off PE's critical path.
    """
    fillers = [[] for _ in range(NIT)]

    def it(qb, hp, kc):
        return (qb * (HC // 2) + hp) * NKC + kc

    # v units: consumer pv(qb0,hp0,kc) emitted at iteration kc+1.
    # (sb0,sc0) is in the prologue; the rest land at iteration kc-1.
    for kc in range(1, NKC):
        fillers[max(0, kc - 1)].append(("v", kc // 4, kc % 4))
    # k-part qk units for hp0 (m=3): scores(qb0,hp0,kc=4j) at iteration 4j.
    for j in range(1, 4):
        fillers[4 * j - 3].append(("qk", j, 3))
    # q-parts for qb0 hp1/hp2 (m=1,2 of sb0) + k-parts m=4,5:
    # scores(qb0,hp1,kc) at iteration 16+kc needs qkT[kc//4][m=4];
    # scores(qb0,hp1,*) needs qkT[0][m=1] at iteration 16.
    fillers[9].append(("qk", 0, 1))
    fillers[12].append(("qk", 0, 4))
    for j in range(1, 4):
        fillers[16 + 4 * j - 3].append(("qk", j, 4))
    fillers[16 + 9].append(("qk", 0, 2))
    fillers[16 + 12].append(("qk", 0, 5))
    for j in range(1, 4):
        fillers[32 + 4 * j - 3].append(("qk", j, 5))
    # q-part for qb1-hp0 (needed at iteration 48)
    fillers[32 + 13].append(("qk", 1, 0))
    # q-parts for later q-blocks: qkT[qb][m] needed at it(qb,hp=m,0).
    fillers[it(1, 0, 13)].append(("qk", 1, 1))
    fillers[it(1, 1, 13)].append(("qk", 1, 2))
    fillers[it(1, 1, 5)].append(("qk", 2, 0))
    fillers[it(2, 0, 5)].append(("qk", 2, 1))
    fillers[it(2, 1, 5)].append(("qk", 2, 2))
    fillers[it(2, 1, 10)].append(("qk", 3, 0))
    fillers[it(3, 0, 5)].append(("qk", 3, 1))
    fillers[it(3, 1, 5)].append(("qk", 3, 2))
    # norms: norm(qb,hp) right after pv(qb,hp,15) (emitted at +1) so the
    # single pv accumulator slot frees quickly (fast fp16 evacuation).
    for qb in range(NQB):
        for hp in range(HC // 2):
            i = it(qb, hp, NKC - 1) + 2
            if i < NIT:
                fillers[i].insert(0, ("norm", qb, hp))
    # output projection of qb inside (qb+1, hp0), after norms released.
    for qb in range(NQB - 1):
        for sc in range(4):
            fillers[it(qb + 1, 0, 6 + 2 * sc)].append(("proj", qb, sc))
    return fillers


def _emit(nc, tc, x_d, wqk_d, wv_d, wp_d, bqk_d, mb_d, mrep_d, y_d):
    ctx_pools = []

    def pool(name, bufs, space="SBUF"):
        p = tc.tile_pool(name=name, bufs=bufs, space=space)
        ctx_pools.append(p)
        return p.__enter__()

    consts = pool("consts", 1)
    store = pool("store", 1)

    ident = consts.tile([128, 128], FP16)
    make_identity(nc, ident[:])

    # weights go over the SWDGE (gpsimd) queue so the x-chunk loads on the
    # sync HWDGE queue aren't serialized behind the weight traffic.
    wqk = consts.tile([128, KCH, QKC], FP16)
    wv = consts.tile([128, KCH, VC], FP16)
    wp = consts.tile([128, VC // 128, E], FP16)
    for k in range(KCH):
        nc.gpsimd.dma_start(wv[:, k, :], wv_d.ap()[k])
    for k in range(KCH):
        nc.gpsimd.dma_start(wqk[:, k, :], wqk_d.ap()[k])
    for t in range(VC // 128):
        nc.gpsimd.dma_start(wp[:, t, :], wp_d.ap()[t])

    bqk = consts.tile([128, QKC // 128], F32)
    nc.scalar.dma_start(bqk[:], bqk_d.ap().rearrange("(c p) -> p c", p=128))
    mb = consts.tile([128, NKC], F32)
    nc.scalar.dma_start(mb[:], mb_d.ap().rearrange("(c p) -> p c", p=128))
    # mask replicated per head: mrep[p, kc, h] = mask[kc*128 + p]
    mrep = consts.tile([128, NKC, HC], FP16)
    nc.scalar.dma_start(
        mrep[:], mrep_d.ap().rearrange("(c h p) -> p c h", p=128, h=HC)
    )

    # qkT store: tile m of 6 holds W-columns m*128..; q cols 0..383 (m 0..2),
    # k cols 384..767 (m 3..5).
    qkT = [
        store.tile([128, QKC // 128, 512], FP16, name=f"qkT{sb}")
        for sb in range(NSB)
    ]
    # v store: per s-block [s-chunk, head, 65] with the mask value (0/1) in
    # column 64 — multiplicative padding mask (masked keys contribute 0 to
    # both the numerator and the softmax denominator).
    vst = [
        store.tile([128, 4, HC, VW], FP16, name=f"vst{sb}") for sb in range(NSB)
    ]
    # attn output (transposed): tile t rows = head dims 2t,2t+1.
    att = store.tile([128, VC // 128, S], FP16)
    # x transposed, kept for the whole run (deferred qk/v projections).
    xts = [store.tile([128, KCH, 512], FP16, name=f"xt{sb}") for sb in range(NSB)]

    # ---- Prologue: load x, transpose all chunks, first qk/v units ----
    xs_p = pool("xs", 8)
    with tc.tile_pool(name="tp", bufs=4, space="PSUM") as tp_p:
        for sg in range(NKC):
            sb, sc = sg // 4, sg % 4
            xs = xs_p.tile([128, E], FP16, tag="xs")
            nc.sync.dma_start(xs[:], x_d.ap()[sg * 128 : (sg + 1) * 128, :])
            tp = tp_p.tile([128, 1024], FP16, tag="tp")
            for k in range(KCH):
                nc.tensor.matmul(
                    tp[:, k * 128 : (k + 1) * 128],
                    xs[:, k * 128 : (k + 1) * 128], ident[:],
                    is_transpose=True,
                    start=(k == 0), stop=(k == KCH - 1),
                )
            nc.vector.tensor_copy(
                xts[sb][:, :, sc * 128 : (sc + 1) * 128],
                tp[:, : KCH * 128].rearrange("p (k f) -> p k f", k=KCH),
            )

    # ---- Fused attention pipeline ----
    # PSUM: st ring 3x2 banks (scores + transient unit tiles) + one 2-bank
    # pv accumulator = exactly 8 banks.
    st_p = pool("st", 3, space="PSUM")   # [128,1024] f32 = 2 banks each
    pv_p = pool("pv", 1, space="PSUM")   # [128,1024] f32 = 2 banks
    pt_p = pool("pt", 3)
    pf_p = pool("pf", 2)
    rs_p = pool("rs", 2)
    bc_p = pool("bc", 2)
    ys_p = pool("ys", 2)

    def unit_v(sb, sc):
        sg = sb * 4 + sc
        u = st_p.tile([128, 1024], F32, tag="st", name=f"va{sg}")
        for k in range(KCH):
            nc.tensor.matmul(
                u[:, 0:VC], xts[sb][:, k, sc * 128 : (sc + 1) * 128],
                wv[:, k, :],
                start=(k == 0), stop=(k == KCH - 1),
            )
        # multiplicative padding mask folded into the V store (the mask
        # value for key row p is a per-partition scalar here).
        nc.vector.tensor_scalar_mul(
            vst[sb][:, sc, :, 0:D],
            u[:, 0:VC].rearrange("p (h d) -> p h d", h=HC),
            mb[:, sg : sg + 1],
        )
        nc.vector.tensor_copy(
            vst[sb][:, sc, :, D : D + 1],
            mrep[:, sg : sg + 1, :].rearrange("p one b -> p b one"),
        )

    def unit_qk(sb, m):
        u = st_p.tile([128, 1024], F32, tag="st", name=f"qk{sb}_{m}")
        for k in range(KCH):
            nc.tensor.matmul(
                u[:, 0:512], wqk[:, k, m * 128 : (m + 1) * 128], xts[sb][:, k, :],
                start=(k == 0), stop=(k == KCH - 1),
            )
        nc.vector.tensor_scalar_add(qkT[sb][:, m, :], u[:, 0:512], bqk[:, m : m + 1])

    def unit_norm(qb, hp, pvs2):
        qs = slice(qb * 512, (qb + 1) * 512)
        # evacuate the pv accumulator to SBUF fp16 right away so the single
        # 2-bank PSUM slot frees for the next head-pair (values are a few
        # thousand at most — far inside fp16 range).
        pvf = pf_p.tile([128, 1024], FP16, tag="pf", name="pvf")
        nc.vector.tensor_copy(pvf[0:VW, :], pvs2[0:VW, :])
        # reciprocal of the softmax denominator row, then rank-1 broadcast
        # to 64 partitions on the (otherwise idle) gpsimd engine.
        rse = rs_p.tile([1, 1024], F32R, tag="rs", name="rse")
        with nc.allow_low_precision(reason="f32r is full width"):
            nc.vector.reciprocal(rse[:], pvf[D : D + 1, :])
        bct = bc_p.tile([D, 1024], F32R, tag="bc", name="bct")
        nc.gpsimd.partition_broadcast(bct[:], rse[:])
        for sub in range(2):
            nc.vector.tensor_tensor(
                att[sub * 64 : sub * 64 + 64, hp, qs],
                pvf[0:D, sub * 512 : (sub + 1) * 512],
                bct[:, sub * 512 : (sub + 1) * 512],
                op=AluOpType.mult,
            )

    def unit_proj(qb, sc):
        sg = qb * 4 + sc
        ys = ys_p.tile([128, E], F32, tag="ys")
        for n0, nw in ((0, 512), (512, 256)):
            ya = st_p.tile([128, 1024], F32, tag="st", name="ya")
            for t in range(VC // 128):
                nc.tensor.matmul(
                    ya[:, :nw],
                    att[:, t, sg * 128 : (sg + 1) * 128],
                    wp[:, t, n0 : n0 + nw],
                    start=(t == 0), stop=(t == VC // 128 - 1),
                )
            nc.vector.tensor_copy(ys[:, n0 : n0 + nw], ya[:, :nw])
        nc.sync.dma_start(y_d.ap()[sg * 128 : (sg + 1) * 128, :], ys[:])

    def emit_unit(u, state):
        kind = u[0]
        if kind == "v":
            unit_v(u[1], u[2])
        elif kind == "qk":
            unit_qk(u[1], u[2])
        elif kind == "norm":
            unit_norm(u[1], u[2], state["pvs2"].pop((u[1], u[2])))
        elif kind == "proj":
            unit_proj(u[1], u[2])

    # prologue units: enough to start (qb0, hp0, kc0).
    unit_qk(0, 0)
    unit_qk(0, 3)
    unit_v(0, 0)

    fillers = _build_schedule()
    state = {"pvs2": {}}
    prev = None  # (pt tile, qb, hp, kc) awaiting its pv matmuls

    def emit_pv(prev):
        pt, qb, hp, kc = prev
        pvs2 = state["pvs2"][(qb, hp)]
        for sub in range(2):
            h = hp * 2 + sub
            nc.tensor.matmul(
                pvs2[0:VW, sub * 512 : (sub + 1) * 512],
                vst[kc // 4][:, kc % 4, h, :],
                pt[:, sub * 512 : (sub + 1) * 512],
                start=(kc == 0), stop=(kc == NKC - 1),
            )

    for i in range(NIT):
        kc = i % NKC
        hp = (i // NKC) % (HC // 2)
        qb = i // (NKC * (HC // 2))
        if kc == 0:
            state["pvs2"][(qb, hp)] = pv_p.tile(
                [128, 1024], F32, tag="pv", name=f"pv{qb}_{hp}"
            )
        # scores: both heads of the pair into one 2-bank f32 tile so one
        # exp instruction (free dim 1024) covers both.
        st = st_p.tile([128, 1024], F32, tag="st")
        for sub in range(2):
            kb, ko = kc // 4, kc % 4
            r0 = sub * 64
            nc.tensor.matmul(
                st[:, sub * 512 : (sub + 1) * 512],
                qkT[kb][r0 : r0 + 64, 3 + hp, ko * 128 : (ko + 1) * 128],
                qkT[qb][r0 : r0 + 64, hp, :],
                start=True, stop=True,
            )
        pt = pt_p.tile([128, 1024], FP16, tag="pt")
        nc.scalar.activation(pt[:], st[:], Act.Exp, scale=0.125)
        for u in fillers[i]:
            emit_unit(u, state)
        if prev is not None:
            emit_pv(prev)
        prev = (pt, qb, hp, kc)
    emit_pv(prev)

    # tail: last head-pair's norm + last q-block projection
    unit_norm(NQB - 1, HC // 2 - 1, state["pvs2"].pop((NQB - 1, HC // 2 - 1)))
    for sc in range(4):
        unit_proj(NQB - 1, sc)

    for p in reversed(ctx_pools):
        p.__exit__(None, None, None)


def make_core_inputs(x, mask, Wqkv, bqkv):
    """Slice full inputs into 8 per-core input maps."""
    x = np.asarray(x, dtype=np.float32)
    mask = np.asarray(mask)
    Wqkv = np.asarray(Wqkv, dtype=np.float32)
    bqkv = np.asarray(bqkv, dtype=np.float32)
    in_maps = []
    for c in range(8):
        b = c // 2
        h0 = (c % 2) * HC
        wq = Wqkv[:, h0 * D : (h0 + HC) * D]
        wk = Wqkv[:, E + h0 * D : E + (h0 + HC) * D]
        wqk = np.concatenate([wq, wk], axis=1).reshape(KCH, 128, QKC)
        wv = Wqkv[:, 2 * E + h0 * D : 2 * E + (h0 + HC) * D].reshape(KCH, 128, VC)
        bqk = np.concatenate(
            [bqkv[h0 * D : (h0 + HC) * D], bqkv[E + h0 * D : E + (h0 + HC) * D]]
        )
        mv = (mask[b, 0, 0, :] != 0).astype(np.float32)  # 1.0 keep, 0.0 drop
        mrep = np.repeat(
            mv.reshape(NKC, 1, 128), HC, axis=1
        ).astype(np.float16).ravel()
        in_maps.append(
            {
                "x": np.ascontiguousarray(x[b].astype(np.float16)),
                "wqk": np.ascontiguousarray(wqk.astype(np.float16)),
                "wv": np.ascontiguousarray(wv.astype(np.float16)),
                "wp": None,  # filled below (needs Wproj)
                "bqk": np.ascontiguousarray(bqk.astype(np.float32)),
                "mb": np.ascontiguousarray(mv),
                "mrep": np.ascontiguousarray(mrep),
            }
        )
    return in_maps


def run(x, mask, Wqkv, bqkv, Wproj, bproj, trace=False, trace_cores=None):
    Wproj = np.asarray(Wproj, dtype=np.float32)
    bproj = np.asarray(bproj, dtype=np.float32)
    bqkv_np = np.asarray(bqkv, dtype=np.float32)
    in_maps = make_core_inputs(x, mask, Wqkv, bqkv_np)
    for c in range(8):
        h0 = (c % 2) * HC
        wp = Wproj[h0 * D : (h0 + HC) * D, :].reshape(VC // 128, 128, E)
        in_maps[c]["wp"] = np.ascontiguousarray(wp.astype(np.float16))

    nc = build_program()
    try:
        res = run_bass_kernel_spmd(
            nc, in_maps, core_ids=list(range(8)), trace=trace,
            trace_cores=trace_cores,
        )
    except Exception:
        # transient device wedge (e.g. NRT_EXEC_UNIT_UNRECOVERABLE) —
        # one retry is usually enough
        res = run_bass_kernel_spmd(
            nc, in_maps, core_ids=list(range(8)), trace=trace,
            trace_cores=trace_cores,
        )
    parts = [res.results[c]["y"] for c in range(8)]

    # host-folded bias: v-bias passes through softmax (weights sum to 1),
    # so y += bv @ Wproj + bproj, applied once per batch row.
    bv = bqkv_np[2 * E : 3 * E]
    bias_row = bv @ Wproj + bproj
    y = np.stack(
        [parts[2 * b] + parts[2 * b + 1] + bias_row for b in range(B)]
    ).astype(np.float32)
    return y, res


def kernel(x, mask, Wqkv, bqkv, Wproj, bproj):
    y, _ = run(x, mask, Wqkv, bqkv, Wproj, bproj, trace=False)
    return y


# revision 18
# speedup vs baseline: 1.1176x; 1.0103x over previous
"""Trainium2 Bass kernel for CodeAttention (B=4, S=2048, E=768, H=12).

Sharding: 8 cores = 4 batches x 2 head-groups (6 heads each).
Each core computes a partial projection output for its batch; the host
sums the two partials per batch and adds the (host-folded) bias row.

v3: fp16 datapath, single fused pipeline.
- The padding mask is folded multiplicatively into the V store (masked
  keys get v=0 AND ones-column=0), so exp needs no per-key-chunk bias.
- The attention kc loop is software-pipelined: scores(i) are emitted
  before pv(i-1) so the scalar engine's exp stream never waits on PE.
- QKV projection / output projection work is emitted as "filler units"
  inside the ACT-paced attention windows, keeping PE busy during exp.
"""

import sys

if "/opt/trn_rl_repo" not in sys.path:
    sys.path.insert(0, "/opt/trn_rl_repo")

import numpy as np

import concourse.bass as bass  # noqa: F401  (engine types referenced via nc)
import concourse.mybir as mybir
import concourse.tile as tile
from concourse import bacc
from concourse.alu_op_type import AluOpType
from concourse.bass_utils import run_bass_kernel_spmd
from concourse.masks import make_identity

F32 = mybir.dt.float32
F32R = mybir.dt.float32r
FP16 = mybir.dt.float16
Act = mybir.ActivationFunctionType

B, S, E, H, D = 4, 2048, 768, 12, 64
HC = 6                    # heads per core
QKC = HC * D * 2          # qk columns per core = 768
VC = HC * D               # v columns per core = 384
KCH = E // 128            # contraction chunks over E = 6
NKC = S // 128            # key chunks = 16
NQB = S // 512            # q blocks of 512 = 4
NSB = S // 512            # s blocks of 512 = 4
VW = D + 1                # v width incl. mask column = 65
NIT = NQB * (HC // 2) * NKC  # flattened attention iterations = 192


def build_program():
    nc = bacc.Bacc("TRN2", target_bir_lowering=False, debug=False, num_devices=8)

    x_d = nc.dram_tensor("x", [S, E], FP16, kind="ExternalInput")
    wqk_d = nc.dram_tensor("wqk", [QKC // 128, KCH, 128, 128], FP16, kind="ExternalInput")
    wv_d = nc.dram_tensor("wv", [KCH, 128, VC], FP16, kind="ExternalInput")
    wp_d = nc.dram_tensor("wp", [VC // 128, 128, E], FP16, kind="ExternalInput")
    bqk_d = nc.dram_tensor("bqk", [QKC], F32, kind="ExternalInput")
    mb_d = nc.dram_tensor("mb", [S], F32, kind="ExternalInput")
    mrep_d = nc.dram_tensor("mrep", [NKC * HC * 128], FP16, kind="ExternalInput")
    y_d = nc.dram_tensor("y", [S, E], F32, kind="ExternalOutput")

    with tile.TileContext(nc) as tc:
        _emit(nc, tc, x_d, wqk_d, wv_d, wp_d, bqk_d, mb_d, mrep_d, y_d)
    nc.compile()
    return nc


def _build_schedule():
    """fillers[i] = list of units to emit inside attention iteration i.

    Units: ("v", sb, sc) v-projection for s-chunk, ("qk", sb, m) qk
    projection m-chunk, ("norm", qb, hp) softmax normalization,
    ("proj", qb, sc) output projection chunk.  Lead-2 scheduling: a unit
    lands >=2 iterations before its first consumer so DVE evacuation of
    the unit's PSUM tile is off PE's critical path.
    """
    fillers = [[] for _ in range(NIT)]

    def it(qb, hp, kc):
        return (qb * (HC // 2) + hp) * NKC + kc

    # transposes for s-chunks 4..15 (sb0 is in the prologue), lead ~6
    # iterations before their first consumer (v unit / qk unit).
    for sg in range(4, NKC):
        fillers[max(0, sg - 6)].append(("tr", sg))
    # v units: consumer pv(qb0,hp0,kc) emitted at iteration kc+1.
    # (sb0,sc0) is in the prologue; the rest land at iteration kc-1.
    for kc in range(1, NKC):
        fillers[max(0, kc - 1)].append(("v", kc // 4, kc % 4))
    # k-part qk units for hp0 (m=3): scores(qb0,hp0,kc=4j) at iteration 4j.
    for j in range(1, 4):
        fillers[4 * j - 2].append(("qk", j, 3))
    # q-parts for qb0 hp1/hp2 (m=1,2 of sb0) + k-parts m=4,5:
    # scores(qb0,hp1,kc) at iteration 16+kc needs qkT[kc//4][m=4];
    # scores(qb0,hp1,*) needs qkT[0][m=1] at iteration 16.
    fillers[13].append(("qk", 0, 1))
    fillers[14].append(("qk", 0, 4))
    for j in range(1, 4):
        fillers[16 + 4 * j - 3].append(("qk", j, 4))
    fillers[16 + 9].append(("qk", 0, 2))
    fillers[16 + 12].append(("qk", 0, 5))
    for j in range(1, 4):
        fillers[32 + 4 * j - 3].append(("qk", j, 5))
    # q-part for qb1-hp0 (needed at iteration 48)
    fillers[32 + 13].append(("qk", 1, 0))
    # q-parts for later q-blocks: qkT[qb][m] needed at it(qb,hp=m,0).
    fillers[it(1, 0, 13)].append(("qk", 1, 1))
    fillers[it(1, 1, 13)].append(("qk", 1, 2))
    fillers[it(1, 1, 5)].append(("qk", 2, 0))
    fillers[it(2, 0, 5)].append(("qk", 2, 1))
    fillers[it(2, 1, 5)].append(("qk", 2, 2))
    fillers[it(2, 1, 10)].append(("qk", 3, 0))
    fillers[it(3, 0, 5)].append(("qk", 3, 1))
    fillers[it(3, 1, 5)].append(("qk", 3, 2))
    # norms: norm(qb,hp) right after pv(qb,hp,15) (emitted at +1) so the
    # single pv accumulator slot frees quickly (fast fp16 evacuation).
    for qb in range(NQB):
        for hp in range(HC // 2):
            i = it(qb, hp, NKC - 1) + 2
            if i < NIT:
                fillers[i].insert(0, ("norm", qb, hp))
    # output projection of qb spread through qb+1 (norms released by then).
    for qb in range(NQB - 1):
        for sc, (hp, kc) in enumerate(((0, 6), (1, 2), (1, 10), (2, 6))):
            fillers[it(qb + 1, hp, kc)].append(("proj", qb, sc))
    return fillers


def _emit(nc, tc, x_d, wqk_d, wv_d, wp_d, bqk_d, mb_d, mrep_d, y_d):
    ctx_pools = []

    def pool(name, bufs, space="SBUF"):
        p = tc.tile_pool(name=name, bufs=bufs, space=space)
        ctx_pools.append(p)
        return p.__enter__()

    consts = pool("consts", 1)
    store = pool("store", 1)

    ident = consts.tile([128, 128], FP16)
    make_identity(nc, ident[:])

    # weights go over the SWDGE (gpsimd) queue so the x-chunk loads on the
    # sync HWDGE queue aren't serialized behind the weight traffic.
    # wqk arrives per m-chunk, ordered by first use (m0/m3 feed the first
    # attention iterations).
    wqk = consts.tile([128, KCH, QKC], FP16)
    wv = consts.tile([128, KCH, VC], FP16)
    wp = consts.tile([128, VC // 128, E], FP16)

    def load_wqk(m):
        nc.gpsimd.dma_start(
            wqk[:, :, m * 128 : (m + 1) * 128],
            wqk_d.ap()[m].rearrange("k p f -> p k f"),
        )

    load_wqk(0)
    load_wqk(3)
    for k in range(KCH):
        nc.gpsimd.dma_start(wv[:, k, :], wv_d.ap()[k])
    load_wqk(1)
    load_wqk(4)
    load_wqk(2)
    load_wqk(5)
    for t in range(VC // 128):
        nc.gpsimd.dma_start(wp[:, t, :], wp_d.ap()[t])

    bqk = consts.tile([128, QKC // 128], F32)
    nc.scalar.dma_start(bqk[:], bqk_d.ap().rearrange("(c p) -> p c", p=128))
    mb = consts.tile([128, NKC], F32)
    nc.scalar.dma_start(mb[:], mb_d.ap().rearrange("(c p) -> p c", p=128))
    # mask replicated per head: mrep[p, kc, h] = mask[kc*128 + p]
    mrep = consts.tile([128, NKC, HC], FP16)
    nc.scalar.dma_start(
        mrep[:], mrep_d.ap().rearrange("(c h p) -> p c h", p=128, h=HC)
    )

    # qkT store: tile m of 6 holds W-columns m*128..; q cols 0..383 (m 0..2),
    # k cols 384..767 (m 3..5).
    qkT = [
        store.tile([128, QKC // 128, 512], FP16, name=f"qkT{sb}")
        for sb in range(NSB)
    ]
    # v store: per s-block [s-chunk, head, 65] with the mask value (0/1) in
    # column 64 — multiplicative padding mask (masked keys contribute 0 to
    # both the numerator and the softmax denominator).
    vst = [
        store.tile([128, 4, HC, VW], FP16, name=f"vst{sb}") for sb in range(NSB)
    ]
    # attn output (transposed): tile t rows = head dims 2t,2t+1.
    att = store.tile([128, VC // 128, S], FP16)
    # x transposed, kept for the whole run (deferred qk/v projections).
    xts = [store.tile([128, KCH, 512], FP16, name=f"xt{sb}") for sb in range(NSB)]

    # ---- Fused pipeline pools ----
    # PSUM: st ring 3x2 banks (scores + transient unit/transpose tiles) +
    # one 2-bank pv accumulator = exactly 8 banks.
    xs_p = pool("xs", NKC)
    st_p = pool("st", 3, space="PSUM")   # [128,1024] f32 = 2 banks each
    pv_p = pool("pv", 1, space="PSUM")   # [128,1024] f32 = 2 banks
    pt_p = pool("pt", 3)
    pf_p = pool("pf", 2)
    rs_p = pool("rs", 2)
    bc_p = pool("bc", 2)
    ys_p = pool("ys", 2)

    # stage all 16 x-chunk loads up front (one SBUF tile each) so the HWDGE
    # queue streams them back-to-back while PE works.
    xss = []
    for sg in range(NKC):
        xs = xs_p.tile([128, E], FP16, tag="xs", name=f"xs{sg}")
        nc.sync.dma_start(xs[:], x_d.ap()[sg * 128 : (sg + 1) * 128, :])
        xss.append(xs)

    def unit_tr(sg):
        sb, sc = sg // 4, sg % 4
        tp = st_p.tile([128, 1024], FP16, tag="st", name=f"tp{sg}")
        for k in range(KCH):
            nc.tensor.matmul(
                tp[:, k * 128 : (k + 1) * 128],
                xss[sg][:, k * 128 : (k + 1) * 128], ident[:],
                is_transpose=True,
                start=(k == 0), stop=(k == KCH - 1),
            )
        nc.vector.tensor_copy(
            xts[sb][:, :, sc * 128 : (sc + 1) * 128],
            tp[:, : KCH * 128].rearrange("p (k f) -> p k f", k=KCH),
        )

    def unit_v(sb, sc):
        sg = sb * 4 + sc
        u = st_p.tile([128, 1024], F32, tag="st", name=f"va{sg}")
        for k in range(KCH):
            nc.tensor.matmul(
                u[:, 0:VC], xts[sb][:, k, sc * 128 : (sc + 1) * 128],
                wv[:, k, :],
                start=(k == 0), stop=(k == KCH - 1),
            )
        # multiplicative padding mask folded into the V store (the mask
        # value for key row p is a per-partition scalar here).
        nc.vector.tensor_scalar_mul(
            vst[sb][:, sc, :, 0:D],
            u[:, 0:VC].rearrange("p (h d) -> p h d", h=HC),
            mb[:, sg : sg + 1],
        )
        nc.vector.tensor_copy(
            vst[sb][:, sc, :, D : D + 1],
            mrep[:, sg : sg + 1, :].rearrange("p one b -> p b one"),
        )

    def unit_qk(sb, m):
        u = st_p.tile([128, 1024], F32, tag="st", name=f"qk{sb}_{m}")
        for k in range(KCH):
            nc.tensor.matmul(
                u[:, 0:512], wqk[:, k, m * 128 : (m + 1) * 128], xts[sb][:, k, :],
                start=(k == 0), stop=(k == KCH - 1),
            )
        nc.vector.tensor_scalar_add(qkT[sb][:, m, :], u[:, 0:512], bqk[:, m : m + 1])

    def unit_norm(qb, hp, pvs2):
        qs = slice(qb * 512, (qb + 1) * 512)
        # evacuate the pv accumulator to SBUF fp16 right away so the single
        # 2-bank PSUM slot frees for the next head-pair (values are a few
        # thousand at most — far inside fp16 range).
        pvf = pf_p.tile([128, 1024], FP16, tag="pf", name="pvf")
        nc.vector.tensor_copy(pvf[0:VW, :], pvs2[0:VW, :])
        # reciprocal of the softmax denominator row, then rank-1 broadcast
        # to 64 partitions on the (otherwise idle) gpsimd engine.
        rse = rs_p.tile([1, 1024], F32R, tag="rs", name="rse")
        with nc.allow_low_precision(reason="f32r is full width"):
            nc.vector.reciprocal(rse[:], pvf[D : D + 1, :])
        bct = bc_p.tile([D, 1024], F32R, tag="bc", name="bct")
        nc.gpsimd.partition_broadcast(bct[:], rse[:])
        for sub in range(2):
            nc.vector.tensor_tensor(
                att[sub * 64 : sub * 64 + 64, hp, qs],
                pvf[0:D, sub * 512 : (sub + 1) * 512],
                bct[:, sub * 512 : (sub + 1) * 512],
                op=AluOpType.mult,
            )

    def unit_proj(qb, sc):
        sg = qb * 4 + sc
        ys = ys_p.tile([128, E], F32, tag="ys")
        for n0, nw in ((0, 512), (512, 256)):
            ya = st_p.tile([128, 1024], F32, tag="st", name="ya")
            for t in range(VC // 128):
                nc.tensor.matmul(
                    ya[:, :nw],
                    att[:, t, sg * 128 : (sg + 1) * 128],
                    wp[:, t, n0 : n0 + nw],
                    start=(t == 0), stop=(t == VC // 128 - 1),
                )
            nc.vector.tensor_copy(ys[:, n0 : n0 + nw], ya[:, :nw])
        nc.sync.dma_start(y_d.ap()[sg * 128 : (sg + 1) * 128, :], ys[:])

    def emit_unit(u, state):
        kind = u[0]
        if kind == "tr":
            unit_tr(u[1])
        elif kind == "v":
            unit_v(u[1], u[2])
        elif kind == "qk":
            unit_qk(u[1], u[2])
        elif kind == "norm":
            unit_norm(u[1], u[2], state["pvs2"].pop((u[1], u[2])))
        elif kind == "proj":
            unit_proj(u[1], u[2])

    # prologue units: enough to start (qb0, hp0, kc0).
    for sg in range(4):
        unit_tr(sg)
    unit_qk(0, 0)
    unit_qk(0, 3)
    unit_v(0, 0)

    fillers = _build_schedule()
    state = {"pvs2": {}}
    prev = None  # (pt tile, qb, hp, kc) awaiting its pv matmuls

    def emit_pv(prev):
        pt, qb, hp, kc = prev
        pvs2 = state["pvs2"][(qb, hp)]
        for sub in range(2):
            h = hp * 2 + sub
            nc.tensor.matmul(
                pvs2[0:VW, sub * 512 : (sub + 1) * 512],
                vst[kc // 4][:, kc % 4, h, :],
                pt[:, sub * 512 : (sub + 1) * 512],
                start=(kc == 0), stop=(kc == NKC - 1),
            )

    for i in range(NIT):
        kc = i % NKC
        hp = (i // NKC) % (HC // 2)
        qb = i // (NKC * (HC // 2))
        if kc == 0:
            state["pvs2"][(qb, hp)] = pv_p.tile(
                [128, 1024], F32, tag="pv", name=f"pv{qb}_{hp}"
            )
        # scores: both heads of the pair into one 2-bank f32 tile so one
        # exp instruction (free dim 1024) covers both.
        st = st_p.tile([128, 1024], F32, tag="st")
        for sub in range(2):
            kb, ko = kc // 4, kc % 4
            r0 = sub * 64
            nc.tensor.matmul(
                st[:, sub * 512 : (sub + 1) * 512],
                qkT[kb][r0 : r0 + 64, 3 + hp, ko * 128 : (ko + 1) * 128],
                qkT[qb][r0 : r0 + 64, hp, :],
                start=True, stop=True,
            )
        pt = pt_p.tile([128, 1024], FP16, tag="pt")
        nc.scalar.activation(pt[:], st[:], Act.Exp, scale=0.125)
        for u in fillers[i]:
            emit_unit(u, state)
        if prev is not None:
            emit_pv(prev)
        prev = (pt, qb, hp, kc)
    emit_pv(prev)

    # tail: last head-pair's norm + last q-block projection
    unit_norm(NQB - 1, HC // 2 - 1, state["pvs2"].pop((NQB - 1, HC // 2 - 1)))
    for sc in range(4):
        unit_proj(NQB - 1, sc)

    for p in reversed(ctx_pools):
        p.__exit__(None, None, None)


def make_core_inputs(x, mask, Wqkv, bqkv):
    """Slice full inputs into 8 per-core input maps."""
    x = np.asarray(x, dtype=np.float32)
    mask = np.asarray(mask)
    Wqkv = np.asarray(Wqkv, dtype=np.float32)
    bqkv = np.asarray(bqkv, dtype=np.float32)
    in_maps = []
    for c in range(8):
        b = c // 2
        h0 = (c % 2) * HC
        wq = Wqkv[:, h0 * D : (h0 + HC) * D]
        wk = Wqkv[:, E + h0 * D : E + (h0 + HC) * D]
        # [m, k, 128, 128]: per-m-chunk DMA granularity
        wqk = np.concatenate([wq, wk], axis=1).reshape(KCH, 128, QKC // 128, 128)
        wqk = wqk.transpose(2, 0, 1, 3)
        wv = Wqkv[:, 2 * E + h0 * D : 2 * E + (h0 + HC) * D].reshape(KCH, 128, VC)
        bqk = np.concatenate(
            [bqkv[h0 * D : (h0 + HC) * D], bqkv[E + h0 * D : E + (h0 + HC) * D]]
        )
        mv = (mask[b, 0, 0, :] != 0).astype(np.float32)  # 1.0 keep, 0.0 drop
        mrep = np.repeat(
            mv.reshape(NKC, 1, 128), HC, axis=1
        ).astype(np.float16).ravel()
        in_maps.append(
            {
                "x": np.ascontiguousarray(x[b].astype(np.float16)),
                "wqk": np.ascontiguousarray(wqk.astype(np.float16)),
                "wv": np.ascontiguousarray(wv.astype(np.float16)),
                "wp": None,  # filled below (needs Wproj)
                "bqk": np.ascontiguousarray(bqk.astype(np.float32)),
                "mb": np.ascontiguousarray(mv),
                "mrep": np.ascontiguousarray(mrep),
            }
        )
    return in_maps


def run(x, mask, Wqkv, bqkv, Wproj, bproj, trace=False, trace_cores=None):
    Wproj = np.asarray(Wproj, dtype=np.float32)
    bproj = np.asarray(bproj, dtype=np.float32)
    bqkv_np = np.asarray(bqkv, dtype=np.float32)
    in_maps = make_core_inputs(x, mask, Wqkv, bqkv_np)
    for c in range(8):
        h0 = (c % 2) * HC
        wp = Wproj[h0 * D : (h0 + HC) * D, :].reshape(VC // 128, 128, E)
        in_maps[c]["wp"] = np.ascontiguousarray(wp.astype(np.float16))

    nc = build_program()
    try:
        res = run_bass_kernel_spmd(
            nc, in_maps, core_ids=list(range(8)), trace=trace,
            trace_cores=trace_cores,
        )
    except Exception:
        # transient device wedge (e.g. NRT_EXEC_UNIT_UNRECOVERABLE) —
        # one retry is usually enough
        res = run_bass_kernel_spmd(
            nc, in_maps, core_ids=list(range(8)), trace=trace,
            trace_cores=trace_cores,
        )
    parts = [res.results[c]["y"] for c in range(8)]

    # host-folded bias: v-bias passes through softmax (weights sum to 1),
    # so y += bv @ Wproj + bproj, applied once per batch row.
    bv = bqkv_np[2 * E : 3 * E]
    bias_row = bv @ Wproj + bproj
    y = np.stack(
        [parts[2 * b] + parts[2 * b + 1] + bias_row for b in range(B)]
    ).astype(np.float32)
    return y, res


def kernel(x, mask, Wqkv, bqkv, Wproj, bproj):
    y, _ = run(x, mask, Wqkv, bqkv, Wproj, bproj, trace=False)
    return y


# revision 27
# speedup vs baseline: 1.1616x; 1.0394x over previous
"""Trainium2 Bass kernel for CodeAttention (B=4, S=2048, E=768, H=12).

Sharding: 8 cores = 4 batches x 2 head-groups (6 heads each).
Each core computes a partial projection output for its batch; the host
sums the two partials per batch and adds the (host-folded) bias row.

v3: fp16 datapath, single fused pipeline.
- The padding mask is folded multiplicatively into the V store (masked
  keys get v=0 AND ones-column=0), so exp needs no per-key-chunk bias.
- The attention kc loop is software-pipelined: scores(i) are emitted
  before pv(i-1) so the scalar engine's exp stream never waits on PE.
- QKV projection / output projection work is emitted as "filler units"
  inside the ACT-paced attention windows, keeping PE busy during exp.
"""

import sys

if "/opt/trn_rl_repo" not in sys.path:
    sys.path.insert(0, "/opt/trn_rl_repo")

import numpy as np

import concourse.bass as bass  # noqa: F401  (engine types referenced via nc)
import concourse.mybir as mybir
import concourse.tile as tile
from concourse import bacc
from concourse.alu_op_type import AluOpType
from concourse.bass_utils import run_bass_kernel_spmd
from concourse.masks import make_identity

F32 = mybir.dt.float32
F32R = mybir.dt.float32r
FP16 = mybir.dt.float16
Act = mybir.ActivationFunctionType

B, S, E, H, D = 4, 2048, 768, 12, 64
HC = 6                    # heads per core
QKC = HC * D * 2          # qk columns per core = 768
VC = HC * D               # v columns per core = 384
KCH = E // 128            # contraction chunks over E = 6
NKC = S // 128            # key chunks = 16
NQB = S // 512            # q blocks of 512 = 4
NSB = S // 512            # s blocks of 512 = 4
VW = D + 1                # v width incl. mask column = 65
NIT = NQB * (HC // 2) * NKC  # flattened attention iterations = 192


def build_program():
    nc = bacc.Bacc("TRN2", target_bir_lowering=False, debug=False, num_devices=8)

    x_d = nc.dram_tensor("x", [S, E], FP16, kind="ExternalInput")
    wqk_d = nc.dram_tensor("wqk", [QKC // 128, KCH, 128, 128], FP16, kind="ExternalInput")
    wv_d = nc.dram_tensor("wv", [KCH, 128, VC], FP16, kind="ExternalInput")
    wp_d = nc.dram_tensor("wp", [VC // 128, 128, E], FP16, kind="ExternalInput")
    bqk_d = nc.dram_tensor("bqk", [QKC], F32, kind="ExternalInput")
    mb_d = nc.dram_tensor("mb", [S], F32, kind="ExternalInput")
    mrep_d = nc.dram_tensor("mrep", [NKC * HC * 128], FP16, kind="ExternalInput")
    y_d = nc.dram_tensor("y", [S, E], F32, kind="ExternalOutput")

    with tile.TileContext(nc) as tc:
        _emit(nc, tc, x_d, wqk_d, wv_d, wp_d, bqk_d, mb_d, mrep_d, y_d)
    nc.compile()
    return nc


def _build_schedule():
    """fillers[i] = list of units to emit inside attention iteration i.

    Units: ("v", sb, sc) v-projection for s-chunk, ("qk", sb, m) qk
    projection m-chunk, ("norm", qb, hp) softmax normalization,
    ("proj", qb, sc) output projection chunk.  Lead-2 scheduling: a unit
    lands >=2 iterations before its first consumer so DVE evacuation of
    the unit's PSUM tile is off PE's critical path.
    """
    fillers = [[] for _ in range(NIT)]

    def it(qb, hp, kc):
        return (qb * (HC // 2) + hp) * NKC + kc

    # transposes for s-chunks 4..15 (sb0 is in the prologue), lead ~6
    # iterations before their first consumer (v unit / qk unit).
    for sg in range(4, NKC):
        fillers[max(0, sg - 6)].append(("tr", sg))
    # v units: consumer pv(qb0,hp0,kc) emitted at iteration kc+1.
    # (sb0,sc0) is in the prologue; the rest land at iteration kc-1.
    for kc in range(1, NKC):
        fillers[max(0, kc - 1)].append(("v", kc // 4, kc % 4))
    # k-part qk units for hp0 (m=3): scores(qb0,hp0,kc=4j) at iteration 4j.
    for j in range(1, 4):
        fillers[4 * j - 2].append(("qk", j, 3))
    # q-parts for qb0 hp1/hp2 (m=1,2 of sb0) + k-parts m=4,5:
    # scores(qb0,hp1,kc) at iteration 16+kc needs qkT[kc//4][m=4];
    # scores(qb0,hp1,*) needs qkT[0][m=1] at iteration 16.
    fillers[13].append(("qk", 0, 1))
    fillers[14].append(("qk", 0, 4))
    for j in range(1, 4):
        fillers[16 + 4 * j - 3].append(("qk", j, 4))
    fillers[16 + 9].append(("qk", 0, 2))
    fillers[16 + 12].append(("qk", 0, 5))
    for j in range(1, 4):
        fillers[32 + 4 * j - 3].append(("qk", j, 5))
    # q-part for qb1-hp0 (needed at iteration 48)
    fillers[32 + 13].append(("qk", 1, 0))
    # q-parts for later q-blocks: qkT[qb][m] needed at it(qb,hp=m,0).
    fillers[it(1, 0, 13)].append(("qk", 1, 1))
    fillers[it(1, 1, 13)].append(("qk", 1, 2))
    fillers[it(1, 1, 5)].append(("qk", 2, 0))
    fillers[it(2, 0, 5)].append(("qk", 2, 1))
    fillers[it(2, 1, 5)].append(("qk", 2, 2))
    fillers[it(2, 1, 10)].append(("qk", 3, 0))
    fillers[it(3, 0, 5)].append(("qk", 3, 1))
    fillers[it(3, 1, 5)].append(("qk", 3, 2))
    # norms: norm(qb,hp) right after pv(qb,hp,15) (emitted at +1) so the
    # single pv accumulator slot frees quickly (fast fp16 evacuation).
    for qb in range(NQB):
        for hp in range(HC // 2):
            i = it(qb, hp, NKC - 1) + 2
            if i < NIT:
                fillers[i].insert(0, ("norm", qb, hp))
    # output projection of qb spread through qb+1 (norms released by then).
    for qb in range(NQB - 1):
        for sc, (hp, kc) in enumerate(((0, 6), (1, 2), (1, 10), (2, 6))):
            fillers[it(qb + 1, hp, kc)].append(("proj", qb, sc))
    return fillers


def _emit(nc, tc, x_d, wqk_d, wv_d, wp_d, bqk_d, mb_d, mrep_d, y_d):
    ctx_pools = []

    def pool(name, bufs, space="SBUF"):
        p = tc.tile_pool(name=name, bufs=bufs, space=space)
        ctx_pools.append(p)
        return p.__enter__()

    consts = pool("consts", 1)
    store = pool("store", 1)

    ident = consts.tile([128, 128], FP16)
    make_identity(nc, ident[:])
    ones_row_f = consts.tile([1, D], F32)
    nc.vector.memset(ones_row_f[:], 1.0)
    ones_row = consts.tile([1, D], F32R)
    nc.vector.tensor_copy(ones_row[:], ones_row_f[:])

    # weights go over the SWDGE (gpsimd) queue so the x-chunk loads on the
    # sync HWDGE queue aren't serialized behind the weight traffic.
    # wqk arrives per m-chunk, ordered by first use (m0/m3 feed the first
    # attention iterations).
    wqk = consts.tile([128, KCH, QKC], FP16)
    wv = consts.tile([128, KCH, VC], FP16)
    wp = consts.tile([128, VC // 128, E], FP16)

    def load_wqk(m):
        nc.gpsimd.dma_start(
            wqk[:, :, m * 128 : (m + 1) * 128],
            wqk_d.ap()[m].rearrange("k p f -> p k f"),
        )

    load_wqk(0)
    load_wqk(3)
    nc.gpsimd.dma_start(wv[:], wv_d.ap().rearrange("k p f -> p k f"))
    load_wqk(1)
    load_wqk(4)
    load_wqk(2)
    load_wqk(5)
    nc.gpsimd.dma_start(wp[:], wp_d.ap().rearrange("t p f -> p t f"))

    # small per-partition tables arrive pre-transposed from the host so the
    # DMAs are one contiguous run per partition (no element-gather).
    bqk = consts.tile([128, QKC // 128], F32)
    nc.scalar.dma_start(bqk[:], bqk_d.ap().rearrange("(p c) -> p c", p=128))
    mb = consts.tile([128, NKC], F32)
    nc.scalar.dma_start(mb[:], mb_d.ap().rearrange("(p c) -> p c", p=128))
    # mask replicated per head: mrep[p, kc, h] = mask[kc*128 + p]
    mrep = consts.tile([128, NKC, HC], FP16)
    nc.scalar.dma_start(
        mrep[:], mrep_d.ap().rearrange("(p c h) -> p c h", p=128, h=HC)
    )

    # qkT store: tile m of 6 holds W-columns m*128..; q cols 0..383 (m 0..2),
    # k cols 384..767 (m 3..5).
    qkT = [
        store.tile([128, QKC // 128, 512], FP16, name=f"qkT{sb}")
        for sb in range(NSB)
    ]
    # v store: per s-block [s-chunk, head, 65] with the mask value (0/1) in
    # column 64 — multiplicative padding mask (masked keys contribute 0 to
    # both the numerator and the softmax denominator).
    vst = [
        store.tile([128, 4, HC, VW], FP16, name=f"vst{sb}") for sb in range(NSB)
    ]
    # attn output (transposed): tile t rows = head dims 2t,2t+1.
    att = store.tile([128, VC // 128, S], FP16)
    # x transposed, kept for the whole run (deferred qk/v projections).
    xts = [store.tile([128, KCH, 512], FP16, name=f"xt{sb}") for sb in range(NSB)]

    # ---- Fused pipeline pools ----
    # PSUM: st ring 3x2 banks (scores + transient unit/transpose tiles) +
    # one 2-bank pv accumulator = exactly 8 banks.
    xs_p = pool("xs", NKC)
    st_p = pool("st", 3, space="PSUM")   # [128,1024] f32 = 2 banks each
    pv_p = pool("pv", 1, space="PSUM")   # [128,1024] f32 = 2 banks
    pt_p = pool("pt", 3)
    pf_p = pool("pf", 2)
    rs_p = pool("rs", 2)
    bc_p = pool("bc", 2)
    ys_p = pool("ys", 2)

    # stage all 16 x-chunk loads up front (one SBUF tile each) so the HWDGE
    # queue streams them back-to-back while PE works.
    xss = []
    for sg in range(NKC):
        xs = xs_p.tile([128, E], FP16, tag="xs", name=f"xs{sg}")
        nc.sync.dma_start(xs[:], x_d.ap()[sg * 128 : (sg + 1) * 128, :])
        xss.append(xs)

    def unit_tr(sg):
        sb, sc = sg // 4, sg % 4
        tp = st_p.tile([128, 1024], FP16, tag="st", name=f"tp{sg}")
        for k in range(KCH):
            nc.tensor.matmul(
                tp[:, k * 128 : (k + 1) * 128],
                xss[sg][:, k * 128 : (k + 1) * 128], ident[:],
                is_transpose=True,
                start=(k == 0), stop=(k == KCH - 1),
            )
        nc.vector.tensor_copy(
            xts[sb][:, :, sc * 128 : (sc + 1) * 128],
            tp[:, : KCH * 128].rearrange("p (k f) -> p k f", k=KCH),
        )

    def unit_v(sb, sc):
        sg = sb * 4 + sc
        u = st_p.tile([128, 1024], F32, tag="st", name=f"va{sg}")
        for k in range(KCH):
            nc.tensor.matmul(
                u[:, 0:VC], xts[sb][:, k, sc * 128 : (sc + 1) * 128],
                wv[:, k, :],
                start=(k == 0), stop=(k == KCH - 1),
            )
        # multiplicative padding mask folded into the V store (the mask
        # value for key row p is a per-partition scalar here).
        nc.vector.tensor_scalar_mul(
            vst[sb][:, sc, :, 0:D],
            u[:, 0:VC].rearrange("p (h d) -> p h d", h=HC),
            mb[:, sg : sg + 1],
        )
        nc.vector.tensor_copy(
            vst[sb][:, sc, :, D : D + 1],
            mrep[:, sg : sg + 1, :].rearrange("p one b -> p b one"),
        )

    def unit_qk(sb, m):
        u = st_p.tile([128, 1024], F32, tag="st", name=f"qk{sb}_{m}")
        for k in range(KCH):
            nc.tensor.matmul(
                u[:, 0:512], wqk[:, k, m * 128 : (m + 1) * 128], xts[sb][:, k, :],
                start=(k == 0), stop=(k == KCH - 1),
            )
        nc.vector.tensor_scalar_add(qkT[sb][:, m, :], u[:, 0:512], bqk[:, m : m + 1])

    def unit_norm(qb, hp, pvs2):
        qs = slice(qb * 512, (qb + 1) * 512)
        # evacuate the pv accumulator to SBUF fp16 right away so the single
        # 2-bank PSUM slot frees for the next head-pair (values are a few
        # thousand at most — far inside fp16 range).
        pvf = pf_p.tile([128, 1024], FP16, tag="pf", name="pvf")
        nc.vector.tensor_copy(pvf[0:VW, :], pvs2[0:VW, :])
        # reciprocal of the softmax denominator row, then rank-1 broadcast
        # to 64 partitions on the (otherwise idle) gpsimd engine.
        rse = rs_p.tile([1, 1024], F32R, tag="rs", name="rse")
        with nc.allow_low_precision(reason="f32r is full width"):
            nc.vector.reciprocal(rse[:], pvf[D : D + 1, :])
        bct = bc_p.tile([D, 1024], F32R, tag="bc", name="bct")
        nc.gpsimd.partition_broadcast(bct[:], rse[:])
        for sub in range(2):
            nc.vector.tensor_tensor(
                att[sub * 64 : sub * 64 + 64, hp, qs],
                pvf[0:D, sub * 512 : (sub + 1) * 512],
                bct[:, sub * 512 : (sub + 1) * 512],
                op=AluOpType.mult,
            )

    def unit_proj(qb, sc):
        sg = qb * 4 + sc
        ys = ys_p.tile([128, E], F32, tag="ys")
        for n0, nw in ((0, 512), (512, 256)):
            ya = st_p.tile([128, 1024], F32, tag="st", name="ya")
            for t in range(VC // 128):
                nc.tensor.matmul(
                    ya[:, :nw],
                    att[:, t, sg * 128 : (sg + 1) * 128],
                    wp[:, t, n0 : n0 + nw],
                    start=(t == 0), stop=(t == VC // 128 - 1),
                )
            nc.vector.tensor_copy(ys[:, n0 : n0 + nw], ya[:, :nw])
            # per-half store overlaps the DMA with the second half's matmuls
            nc.sync.dma_start(
                y_d.ap()[sg * 128 : (sg + 1) * 128, n0 : n0 + nw],
                ys[:, n0 : n0 + nw],
            )

    def emit_unit(u, state):
        kind = u[0]
        if kind == "tr":
            unit_tr(u[1])
        elif kind == "v":
            unit_v(u[1], u[2])
        elif kind == "qk":
            unit_qk(u[1], u[2])
        elif kind == "norm":
            unit_norm(u[1], u[2], state["pvs2"].pop((u[1], u[2])))
        elif kind == "proj":
            unit_proj(u[1], u[2])

    # prologue units: enough to start (qb0, hp0, kc0).
    for sg in range(4):
        unit_tr(sg)
    unit_qk(0, 0)
    unit_qk(0, 3)
    unit_v(0, 0)

    fillers = _build_schedule()
    state = {"pvs2": {}}
    prev = None  # (pt tile, qb, hp, kc) awaiting its pv matmuls

    def emit_pv(prev):
        pt, qb, hp, kc = prev
        pvs2 = state["pvs2"][(qb, hp)]
        for sub in range(2):
            h = hp * 2 + sub
            nc.tensor.matmul(
                pvs2[0:VW, sub * 512 : (sub + 1) * 512],
                vst[kc // 4][:, kc % 4, h, :],
                pt[:, sub * 512 : (sub + 1) * 512],
                start=(kc == 0), stop=(kc == NKC - 1),
            )

    for i in range(NIT):
        kc = i % NKC
        hp = (i // NKC) % (HC // 2)
        qb = i // (NKC * (HC // 2))
        if kc == 0:
            state["pvs2"][(qb, hp)] = pv_p.tile(
                [128, 1024], F32, tag="pv", name=f"pv{qb}_{hp}"
            )
        # scores: both heads of the pair into one 2-bank f32 tile so one
        # exp instruction (free dim 1024) covers both.
        st = st_p.tile([128, 1024], F32, tag="st")
        for sub in range(2):
            kb, ko = kc // 4, kc % 4
            r0 = sub * 64
            nc.tensor.matmul(
                st[:, sub * 512 : (sub + 1) * 512],
                qkT[kb][r0 : r0 + 64, 3 + hp, ko * 128 : (ko + 1) * 128],
                qkT[qb][r0 : r0 + 64, hp, :],
                start=True, stop=True,
            )
        pt = pt_p.tile([128, 1024], FP16, tag="pt")
        nc.scalar.activation(pt[:], st[:], Act.Exp, scale=0.125)
        for u in fillers[i]:
            emit_unit(u, state)
        if prev is not None:
            emit_pv(prev)
        prev = (pt, qb, hp, kc)
    emit_pv(prev)

    # tail: last head-pair's norm with the shortest possible chain — read
    # the pv accumulator in place and broadcast on PE (idle by now).
    pvs2 = state["pvs2"].pop((NQB - 1, HC // 2 - 1))
    qs = slice((NQB - 1) * 512, NQB * 512)
    rse = rs_p.tile([1, 1024], F32R, tag="rs", name="rse_t")
    with nc.allow_low_precision(reason="f32r is full width"):
        nc.vector.reciprocal(rse[:], pvs2[D : D + 1, :])
    pvf = pf_p.tile([128, 1024], FP16, tag="pf", name="pvf_t")
    nc.vector.tensor_copy(pvf[0:D, :], pvs2[0:D, :])
    bcp = st_p.tile([128, 1024], F32, tag="st", name="bcp")
    for sub in range(2):
        nc.tensor.matmul(
            bcp[0:D, sub * 512 : (sub + 1) * 512],
            ones_row[:], rse[:, sub * 512 : (sub + 1) * 512],
            start=True, stop=True,
        )
    for sub in range(2):
        nc.vector.tensor_tensor(
            att[sub * 64 : sub * 64 + 64, HC // 2 - 1, qs],
            pvf[0:D, sub * 512 : (sub + 1) * 512],
            bcp[0:D, sub * 512 : (sub + 1) * 512],
            op=AluOpType.mult,
        )
    for sc in range(4):
        unit_proj(NQB - 1, sc)

    for p in reversed(ctx_pools):
        p.__exit__(None, None, None)


def make_core_inputs(x, mask, Wqkv, bqkv):
    """Slice full inputs into 8 per-core input maps."""
    x = np.asarray(x, dtype=np.float32)
    mask = np.asarray(mask)
    Wqkv = np.asarray(Wqkv, dtype=np.float32)
    bqkv = np.asarray(bqkv, dtype=np.float32)
    in_maps = []
    for c in range(8):
        b = c // 2
        h0 = (c % 2) * HC
        wq = Wqkv[:, h0 * D : (h0 + HC) * D]
        wk = Wqkv[:, E + h0 * D : E + (h0 + HC) * D]
        # [m, k, 128, 128]: per-m-chunk DMA granularity
        wqk = np.concatenate([wq, wk], axis=1).reshape(KCH, 128, QKC // 128, 128)
        wqk = wqk.transpose(2, 0, 1, 3)
        wv = Wqkv[:, 2 * E + h0 * D : 2 * E + (h0 + HC) * D].reshape(KCH, 128, VC)
        bqk = np.concatenate(
            [bqkv[h0 * D : (h0 + HC) * D], bqkv[E + h0 * D : E + (h0 + HC) * D]]
        )
        mv = (mask[b, 0, 0, :] != 0).astype(np.float32)  # 1.0 keep, 0.0 drop
        mb_t = mv.reshape(NKC, 128).T  # [p, c]
        mrep = np.repeat(mb_t[:, :, None], HC, axis=2)  # [p, c, h]
        in_maps.append(
            {
                "x": np.ascontiguousarray(x[b].astype(np.float16)),
                "wqk": np.ascontiguousarray(wqk.astype(np.float16)),
                "wv": np.ascontiguousarray(wv.astype(np.float16)),
                "wp": None,  # filled below (needs Wproj)
                "bqk": np.ascontiguousarray(
                    bqk.reshape(QKC // 128, 128).T.astype(np.float32).ravel()
                ),
                "mb": np.ascontiguousarray(mb_t.astype(np.float32).ravel()),
                "mrep": np.ascontiguousarray(mrep.astype(np.float16).ravel()),
            }
        )
    return in_maps


def run(x, mask, Wqkv, bqkv, Wproj, bproj, trace=False, trace_cores=None):
    Wproj = np.asarray(Wproj, dtype=np.float32)
    bproj = np.asarray(bproj, dtype=np.float32)
    bqkv_np = np.asarray(bqkv, dtype=np.float32)
    in_maps = make_core_inputs(x, mask, Wqkv, bqkv_np)
    for c in range(8):
        h0 = (c % 2) * HC
        wp = Wproj[h0 * D : (h0 + HC) * D, :].reshape(VC // 128, 128, E)
        in_maps[c]["wp"] = np.ascontiguousarray(wp.astype(np.float16))

    nc = build_program()
    try:
        res = run_bass_kernel_spmd(
            nc, in_maps, core_ids=list(range(8)), trace=trace,
            trace_cores=trace_cores,
        )
    except Exception:
        # transient device wedge (e.g. NRT_EXEC_UNIT_UNRECOVERABLE) —
        # one retry is usually enough
        res = run_bass_kernel_spmd(
            nc, in_maps, core_ids=list(range(8)), trace=trace,
            trace_cores=trace_cores,
        )
    parts = [res.results[c]["y"] for c in range(8)]

    # host-folded bias: v-bias passes through softmax (weights sum to 1),
    # so y += bv @ Wproj + bproj, applied once per batch row.
    bv = bqkv_np[2 * E : 3 * E]
    bias_row = bv @ Wproj + bproj
    y = np.stack(
        [parts[2 * b] + parts[2 * b + 1] + bias_row for b in range(B)]
    ).astype(np.float32)
    return y, res


def kernel(x, mask, Wqkv, bqkv, Wproj, bproj):
    y, _ = run(x, mask, Wqkv, bqkv, Wproj, bproj, trace=False)
    return y


# revision 35
# speedup vs baseline: 1.1712x; 1.0083x over previous
"""Trainium2 Bass kernel for CodeAttention (B=4, S=2048, E=768, H=12).

Sharding: 8 cores = 4 batches x 2 head-groups (6 heads each).
Each core computes a partial projection output for its batch; the host
sums the two partials per batch and adds the (host-folded) bias row.

v3: fp16 datapath, single fused pipeline.
- The padding mask is folded multiplicatively into the V store (masked
  keys get v=0 AND ones-column=0), so exp needs no per-key-chunk bias.
- The attention kc loop is software-pipelined: scores(i) are emitted
  before pv(i-1) so the scalar engine's exp stream never waits on PE.
- QKV projection / output projection work is emitted as "filler units"
  inside the ACT-paced attention windows, keeping PE busy during exp.
"""

import sys

if "/opt/trn_rl_repo" not in sys.path:
    sys.path.insert(0, "/opt/trn_rl_repo")

import numpy as np

import concourse.bass as bass  # noqa: F401  (engine types referenced via nc)
import concourse.mybir as mybir
import concourse.tile as tile
from concourse import bacc
from concourse.alu_op_type import AluOpType
from concourse.bass_utils import run_bass_kernel_spmd
from concourse.masks import make_identity

F32 = mybir.dt.float32
F32R = mybir.dt.float32r
FP16 = mybir.dt.float16
Act = mybir.ActivationFunctionType

B, S, E, H, D = 4, 2048, 768, 12, 64
HC = 6                    # heads per core
QKC = HC * D * 2          # qk columns per core = 768
VC = HC * D               # v columns per core = 384
KCH = E // 128            # contraction chunks over E = 6
NKC = S // 128            # key chunks = 16
NQB = S // 512            # q blocks of 512 = 4
NSB = S // 512            # s blocks of 512 = 4
VW = D + 1                # v width incl. mask column = 65
NIT = NQB * (HC // 2) * NKC  # flattened attention iterations = 192


def build_program():
    nc = bacc.Bacc("TRN2", target_bir_lowering=False, debug=False, num_devices=8)

    x_d = nc.dram_tensor("x", [S, E], FP16, kind="ExternalInput")
    wqk_d = nc.dram_tensor("wqk", [QKC // 128, KCH, 128, 128], FP16, kind="ExternalInput")
    wv_d = nc.dram_tensor("wv", [KCH, 128, VC], FP16, kind="ExternalInput")
    wp_d = nc.dram_tensor("wp", [VC // 128, 128, E], FP16, kind="ExternalInput")
    bqk_d = nc.dram_tensor("bqk", [QKC], F32, kind="ExternalInput")
    mb_d = nc.dram_tensor("mb", [S], F32, kind="ExternalInput")
    mrep_d = nc.dram_tensor("mrep", [NKC * HC * 128], FP16, kind="ExternalInput")
    y_d = nc.dram_tensor("y", [S, E], F32, kind="ExternalOutput")

    with tile.TileContext(nc) as tc:
        _emit(nc, tc, x_d, wqk_d, wv_d, wp_d, bqk_d, mb_d, mrep_d, y_d)
    nc.compile()
    return nc


def _build_schedule():
    """fillers[i] = list of units to emit inside attention iteration i.

    Units: ("v", sb, sc) v-projection for s-chunk, ("qk", sb, m) qk
    projection m-chunk, ("norm", qb, hp) softmax normalization,
    ("proj", qb, sc) output projection chunk.  Lead-2 scheduling: a unit
    lands >=2 iterations before its first consumer so DVE evacuation of
    the unit's PSUM tile is off PE's critical path.
    """
    fillers = [[] for _ in range(NIT)]

    def it(qb, hp, kc):
        return (qb * (HC // 2) + hp) * NKC + kc

    # transposes for s-chunks 4..15 (sb0 is in the prologue), lead ~6
    # iterations before their first consumer (v unit / qk unit).
    for sg in range(4, NKC):
        fillers[max(0, sg - 6)].append(("tr", sg))
    # v units: consumer pv(qb0,hp0,kc) emitted at iteration kc+1.
    # (sb0,sc0) is in the prologue; the rest land at iteration kc-1.
    for kc in range(1, NKC):
        fillers[max(0, kc - 1)].append(("v", kc // 4, kc % 4))
    # k-part qk units for hp0 (m=3): scores(qb0,hp0,kc=4j) at iteration 4j.
    for j in range(1, 4):
        fillers[4 * j - 2].append(("qk", j, 3))
    # q-parts for qb0 hp1/hp2 (m=1,2 of sb0) + k-parts m=4,5:
    # scores(qb0,hp1,kc) at iteration 16+kc needs qkT[kc//4][m=4];
    # scores(qb0,hp1,*) needs qkT[0][m=1] at iteration 16.
    fillers[13].append(("qk", 0, 1))
    fillers[14].append(("qk", 0, 4))
    for j in range(1, 4):
        fillers[16 + 4 * j - 3].append(("qk", j, 4))
    fillers[16 + 9].append(("qk", 0, 2))
    fillers[16 + 12].append(("qk", 0, 5))
    for j in range(1, 4):
        fillers[32 + 4 * j - 3].append(("qk", j, 5))
    # q-part for qb1-hp0 (needed at iteration 48)
    fillers[32 + 13].append(("qk", 1, 0))
    # q-parts for later q-blocks: qkT[qb][m] needed at it(qb,hp=m,0).
    fillers[it(1, 0, 13)].append(("qk", 1, 1))
    fillers[it(1, 1, 13)].append(("qk", 1, 2))
    fillers[it(1, 1, 5)].append(("qk", 2, 0))
    fillers[it(2, 0, 5)].append(("qk", 2, 1))
    fillers[it(2, 1, 5)].append(("qk", 2, 2))
    fillers[it(2, 1, 10)].append(("qk", 3, 0))
    fillers[it(3, 0, 5)].append(("qk", 3, 1))
    fillers[it(3, 1, 5)].append(("qk", 3, 2))
    # norms: norm(qb,hp) right after pv(qb,hp,15) (emitted at +2 with the
    # depth-2 pv pipeline) so the single pv slot frees quickly.
    for qb in range(NQB):
        for hp in range(HC // 2):
            i = it(qb, hp, NKC - 1) + 3
            if i < NIT:
                fillers[i].insert(0, ("norm", qb, hp))
    # output projection of qb spread through qb+1 (norms released by then).
    for qb in range(NQB - 1):
        for sc, (hp, kc) in enumerate(((0, 6), (1, 2), (1, 10), (2, 6))):
            fillers[it(qb + 1, hp, kc)].append(("proj", qb, sc))
    return fillers


def _emit(nc, tc, x_d, wqk_d, wv_d, wp_d, bqk_d, mb_d, mrep_d, y_d):
    ctx_pools = []

    def pool(name, bufs, space="SBUF"):
        p = tc.tile_pool(name=name, bufs=bufs, space=space)
        ctx_pools.append(p)
        return p.__enter__()

    consts = pool("consts", 1)
    store = pool("store", 1)

    ident = consts.tile([128, 128], FP16)
    make_identity(nc, ident[:])
    ones_row_f = consts.tile([1, D], F32)
    nc.vector.memset(ones_row_f[:], 1.0)
    ones_row = consts.tile([1, D], F32R)
    nc.vector.tensor_copy(ones_row[:], ones_row_f[:])

    # weights go over the SWDGE (gpsimd) queue so the x-chunk loads on the
    # sync HWDGE queue aren't serialized behind the weight traffic.
    # wqk arrives per m-chunk, ordered by first use (m0/m3 feed the first
    # attention iterations).
    wqk = consts.tile([128, KCH, QKC], FP16)
    wv = consts.tile([128, KCH, VC], FP16)
    wp = consts.tile([128, VC // 128, E], FP16)

    def load_wqk(m):
        nc.gpsimd.dma_start(
            wqk[:, :, m * 128 : (m + 1) * 128],
            wqk_d.ap()[m].rearrange("k p f -> p k f"),
        )

    load_wqk(0)
    nc.gpsimd.dma_start(wv[:], wv_d.ap().rearrange("k p f -> p k f"))
    load_wqk(3)
    load_wqk(1)
    load_wqk(4)
    load_wqk(2)
    load_wqk(5)
    nc.gpsimd.dma_start(wp[:], wp_d.ap().rearrange("t p f -> p t f"))

    # small per-partition tables arrive pre-transposed from the host so the
    # DMAs are one contiguous run per partition (no element-gather).
    bqk = consts.tile([128, QKC // 128], F32)
    nc.scalar.dma_start(bqk[:], bqk_d.ap().rearrange("(p c) -> p c", p=128))
    mb = consts.tile([128, NKC], F32)
    nc.scalar.dma_start(mb[:], mb_d.ap().rearrange("(p c) -> p c", p=128))
    # mask replicated per head: mrep[p, kc, h] = mask[kc*128 + p]
    mrep = consts.tile([128, NKC, HC], FP16)
    nc.scalar.dma_start(
        mrep[:], mrep_d.ap().rearrange("(p c h) -> p c h", p=128, h=HC)
    )

    # qkT store: tile m of 6 holds W-columns m*128..; q cols 0..383 (m 0..2),
    # k cols 384..767 (m 3..5).
    qkT = [
        store.tile([128, QKC // 128, 512], FP16, name=f"qkT{sb}")
        for sb in range(NSB)
    ]
    # v store: per s-block [s-chunk, head, 65] with the mask value (0/1) in
    # column 64 — multiplicative padding mask (masked keys contribute 0 to
    # both the numerator and the softmax denominator).
    vst = [
        store.tile([128, 4, HC, VW], FP16, name=f"vst{sb}") for sb in range(NSB)
    ]
    # attn output (transposed): tile t rows = head dims 2t,2t+1.
    att = store.tile([128, VC // 128, S], FP16)
    # x transposed, kept for the whole run (deferred qk/v projections).
    xts = [store.tile([128, KCH, 512], FP16, name=f"xt{sb}") for sb in range(NSB)]

    # ---- Fused pipeline pools ----
    # PSUM: st ring 3x2 banks (scores + transient unit/transpose tiles) +
    # one 2-bank pv accumulator = exactly 8 banks.
    xs_p = pool("xs", NKC)
    st_p = pool("st", 3, space="PSUM")   # [128,1024] f32 = 2 banks each
    pv_p = pool("pv", 1, space="PSUM")   # [128,1024] f32 = 2 banks
    pt_p = pool("pt", 4)
    pf_p = pool("pf", 2)
    rs_p = pool("rs", 2)
    bc_p = pool("bc", 2)
    ys_p = pool("ys", 2)

    # stage all 16 x-chunk loads up front (one SBUF tile each), alternating
    # between the two HWDGE queues so chunks land twice as fast.
    xss = []
    for sg in range(NKC):
        xs = xs_p.tile([128, E], FP16, tag="xs", name=f"xs{sg}")
        q = nc.sync if sg % 2 == 0 else nc.scalar
        q.dma_start(xs[:], x_d.ap()[sg * 128 : (sg + 1) * 128, :])
        xss.append(xs)

    def unit_tr(sg):
        sb, sc = sg // 4, sg % 4
        tp = st_p.tile([128, 1024], FP16, tag="st", name=f"tp{sg}")
        for k in range(KCH):
            nc.tensor.matmul(
                tp[:, k * 128 : (k + 1) * 128],
                xss[sg][:, k * 128 : (k + 1) * 128], ident[:],
                is_transpose=True,
                start=(k == 0), stop=(k == KCH - 1),
            )
        nc.vector.tensor_copy(
            xts[sb][:, :, sc * 128 : (sc + 1) * 128],
            tp[:, : KCH * 128].rearrange("p (k f) -> p k f", k=KCH),
        )

    def unit_v(sb, sc):
        sg = sb * 4 + sc
        u = st_p.tile([128, 1024], F32, tag="st", name=f"va{sg}")
        for k in range(KCH):
            nc.tensor.matmul(
                u[:, 0:VC], xts[sb][:, k, sc * 128 : (sc + 1) * 128],
                wv[:, k, :],
                start=(k == 0), stop=(k == KCH - 1),
            )
        # multiplicative padding mask folded into the V store (the mask
        # value for key row p is a per-partition scalar here).
        nc.vector.tensor_scalar_mul(
            vst[sb][:, sc, :, 0:D],
            u[:, 0:VC].rearrange("p (h d) -> p h d", h=HC),
            mb[:, sg : sg + 1],
        )
        nc.vector.tensor_copy(
            vst[sb][:, sc, :, D : D + 1],
            mrep[:, sg : sg + 1, :].rearrange("p one b -> p b one"),
        )

    def unit_qk(sb, m):
        u = st_p.tile([128, 1024], F32, tag="st", name=f"qk{sb}_{m}")
        for k in range(KCH):
            nc.tensor.matmul(
                u[:, 0:512], wqk[:, k, m * 128 : (m + 1) * 128], xts[sb][:, k, :],
                start=(k == 0), stop=(k == KCH - 1),
            )
        nc.vector.tensor_scalar_add(qkT[sb][:, m, :], u[:, 0:512], bqk[:, m : m + 1])

    def unit_norm(qb, hp, pvs2):
        qs = slice(qb * 512, (qb + 1) * 512)
        # evacuate the pv accumulator to SBUF fp16 right away so the single
        # 2-bank PSUM slot frees for the next head-pair (values are a few
        # thousand at most — far inside fp16 range).
        pvf = pf_p.tile([128, 1024], FP16, tag="pf", name="pvf")
        nc.vector.tensor_copy(pvf[0:VW, :], pvs2[0:VW, :])
        # reciprocal of the softmax denominator row, then rank-1 broadcast
        # to 64 partitions on the (otherwise idle) gpsimd engine.
        rse = rs_p.tile([1, 1024], F32R, tag="rs", name="rse")
        with nc.allow_low_precision(reason="f32r is full width"):
            nc.vector.reciprocal(rse[:], pvf[D : D + 1, :])
        bct = bc_p.tile([D, 1024], F32R, tag="bc", name="bct")
        nc.gpsimd.partition_broadcast(bct[:], rse[:])
        for sub in range(2):
            nc.vector.tensor_tensor(
                att[sub * 64 : sub * 64 + 64, hp, qs],
                pvf[0:D, sub * 512 : (sub + 1) * 512],
                bct[:, sub * 512 : (sub + 1) * 512],
                op=AluOpType.mult,
            )

    def unit_proj(qb, sc):
        sg = qb * 4 + sc
        ys = ys_p.tile([128, E], F32, tag="ys")
        for n0, nw in ((0, 512), (512, 256)):
            ya = st_p.tile([128, 1024], F32, tag="st", name="ya")
            for t in range(VC // 128):
                nc.tensor.matmul(
                    ya[:, :nw],
                    att[:, t, sg * 128 : (sg + 1) * 128],
                    wp[:, t, n0 : n0 + nw],
                    start=(t == 0), stop=(t == VC // 128 - 1),
                )
            nc.vector.tensor_copy(ys[:, n0 : n0 + nw], ya[:, :nw])
            # per-half store overlaps the DMA with the second half's matmuls
            nc.sync.dma_start(
                y_d.ap()[sg * 128 : (sg + 1) * 128, n0 : n0 + nw],
                ys[:, n0 : n0 + nw],
            )

    def emit_unit(u, state):
        kind = u[0]
        if kind == "tr":
            unit_tr(u[1])
        elif kind == "v":
            unit_v(u[1], u[2])
        elif kind == "qk":
            unit_qk(u[1], u[2])
        elif kind == "norm":
            unit_norm(u[1], u[2], state["pvs2"].pop((u[1], u[2])))
        elif kind == "proj":
            unit_proj(u[1], u[2])

    # prologue units: enough to start (qb0, hp0, kc0).
    for sg in range(4):
        unit_tr(sg)
    unit_qk(0, 0)
    unit_v(0, 0)
    unit_qk(0, 3)

    fillers = _build_schedule()
    state = {"pvs2": {}}
    pending = []  # (pt tile, qb, hp, kc) awaiting pv matmuls (depth-2 pipe)

    def emit_pv(prev):
        pt, qb, hp, kc = prev
        pvs2 = state["pvs2"][(qb, hp)]
        for sub in range(2):
            h = hp * 2 + sub
            nc.tensor.matmul(
                pvs2[0:VW, sub * 512 : (sub + 1) * 512],
                vst[kc // 4][:, kc % 4, h, :],
                pt[:, sub * 512 : (sub + 1) * 512],
                start=(kc == 0), stop=(kc == NKC - 1),
            )

    for i in range(NIT):
        kc = i % NKC
        hp = (i // NKC) % (HC // 2)
        qb = i // (NKC * (HC // 2))
        if kc == 0:
            state["pvs2"][(qb, hp)] = pv_p.tile(
                [128, 1024], F32, tag="pv", name=f"pv{qb}_{hp}"
            )
        # scores: both heads of the pair into one 2-bank f32 tile so one
        # exp instruction (free dim 1024) covers both.
        st = st_p.tile([128, 1024], F32, tag="st")
        for sub in range(2):
            kb, ko = kc // 4, kc % 4
            r0 = sub * 64
            nc.tensor.matmul(
                st[:, sub * 512 : (sub + 1) * 512],
                qkT[kb][r0 : r0 + 64, 3 + hp, ko * 128 : (ko + 1) * 128],
                qkT[qb][r0 : r0 + 64, hp, :],
                start=True, stop=True,
            )
        pt = pt_p.tile([128, 1024], FP16, tag="pt")
        nc.scalar.activation(pt[:], st[:], Act.Exp, scale=0.125)
        for u in fillers[i]:
            emit_unit(u, state)
        if len(pending) >= 2:
            emit_pv(pending.pop(0))
        pending.append((pt, qb, hp, kc))
    for p in pending:
        emit_pv(p)

    # tail: last head-pair's norm with the shortest possible chain — read
    # the pv accumulator in place and broadcast on PE (idle by now).
    pvs2 = state["pvs2"].pop((NQB - 1, HC // 2 - 1))
    qs = slice((NQB - 1) * 512, NQB * 512)
    rse = rs_p.tile([1, 1024], F32R, tag="rs", name="rse_t")
    with nc.allow_low_precision(reason="f32r is full width"):
        nc.vector.reciprocal(rse[:], pvs2[D : D + 1, :])
    pvf = pf_p.tile([128, 1024], FP16, tag="pf", name="pvf_t")
    nc.vector.tensor_copy(pvf[0:D, :], pvs2[0:D, :])
    # start the last q-block's projection on the heads that are already
    # normalized (t=0,1) while the reciprocal/broadcast chain drains; the
    # t=2 accumulation step joins after the final normalize below.
    yas, yss = [], []
    for sc in range(2):
        sg = (NQB - 1) * 4 + sc
        ya = st_p.tile([128, 1024], F32, tag="st", name=f"yat{sc}")
        for n0, nw in ((0, 512), (512, 256)):
            for t in range(2):
                nc.tensor.matmul(
                    ya[:, n0 : n0 + nw] if n0 == 0 else ya[:, 512 : 512 + nw],
                    att[:, t, sg * 128 : (sg + 1) * 128],
                    wp[:, t, n0 : n0 + nw],
                    start=(t == 0), stop=False,
                )
        yas.append(ya)
    bcp = st_p.tile([128, 1024], F32, tag="st", name="bcp")
    for sub in range(2):
        nc.tensor.matmul(
            bcp[0:D, sub * 512 : (sub + 1) * 512],
            ones_row[:], rse[:, sub * 512 : (sub + 1) * 512],
            start=True, stop=True,
        )
    for sub in range(2):
        nc.vector.tensor_tensor(
            att[sub * 64 : sub * 64 + 64, HC // 2 - 1, qs],
            pvf[0:D, sub * 512 : (sub + 1) * 512],
            bcp[0:D, sub * 512 : (sub + 1) * 512],
            op=AluOpType.mult,
        )
    for sc in range(2):
        sg = (NQB - 1) * 4 + sc
        ya = yas[sc]
        ys = ys_p.tile([128, E], F32, tag="ys")
        for n0, nw in ((0, 512), (512, 256)):
            nc.tensor.matmul(
                ya[:, n0 : n0 + nw],
                att[:, 2, sg * 128 : (sg + 1) * 128],
                wp[:, 2, n0 : n0 + nw],
                start=False, stop=True,
            )
            nc.vector.tensor_copy(ys[:, n0 : n0 + nw], ya[:, n0 : n0 + nw])
            nc.sync.dma_start(
                y_d.ap()[sg * 128 : (sg + 1) * 128, n0 : n0 + nw],
                ys[:, n0 : n0 + nw],
            )
    for sc in range(2, 4):
        unit_proj(NQB - 1, sc)

    for p in reversed(ctx_pools):
        p.__exit__(None, None, None)


def make_core_inputs(x, mask, Wqkv, bqkv):
    """Slice full inputs into 8 per-core input maps."""
    x = np.asarray(x, dtype=np.float32)
    mask = np.asarray(mask)
    Wqkv = np.asarray(Wqkv, dtype=np.float32)
    bqkv = np.asarray(bqkv, dtype=np.float32)
    in_maps = []
    for c in range(8):
        b = c // 2
        h0 = (c % 2) * HC
        wq = Wqkv[:, h0 * D : (h0 + HC) * D]
        wk = Wqkv[:, E + h0 * D : E + (h0 + HC) * D]
        # [m, k, 128, 128]: per-m-chunk DMA granularity
        wqk = np.concatenate([wq, wk], axis=1).reshape(KCH, 128, QKC // 128, 128)
        wqk = wqk.transpose(2, 0, 1, 3)
        wv = Wqkv[:, 2 * E + h0 * D : 2 * E + (h0 + HC) * D].reshape(KCH, 128, VC)
        bqk = np.concatenate(
            [bqkv[h0 * D : (h0 + HC) * D], bqkv[E + h0 * D : E + (h0 + HC) * D]]
        )
        mv = (mask[b, 0, 0, :] != 0).astype(np.float32)  # 1.0 keep, 0.0 drop
        mb_t = mv.reshape(NKC, 128).T  # [p, c]
        mrep = np.repeat(mb_t[:, :, None], HC, axis=2)  # [p, c, h]
        in_maps.append(
            {
                "x": np.ascontiguousarray(x[b].astype(np.float16)),
                "wqk": np.ascontiguousarray(wqk.astype(np.float16)),
                "wv": np.ascontiguousarray(wv.astype(np.float16)),
                "wp": None,  # filled below (needs Wproj)
                "bqk": np.ascontiguousarray(
                    bqk.reshape(QKC // 128, 128).T.astype(np.float32).ravel()
                ),
                "mb": np.ascontiguousarray(mb_t.astype(np.float32).ravel()),
                "mrep": np.ascontiguousarray(mrep.astype(np.float16).ravel()),
            }
        )
    return in_maps


def run(x, mask, Wqkv, bqkv, Wproj, bproj, trace=False, trace_cores=None):
    Wproj = np.asarray(Wproj, dtype=np.float32)
    bproj = np.asarray(bproj, dtype=np.float32)
    bqkv_np = np.asarray(bqkv, dtype=np.float32)
    in_maps = make_core_inputs(x, mask, Wqkv, bqkv_np)
    for c in range(8):
        h0 = (c % 2) * HC
        wp = Wproj[h0 * D : (h0 + HC) * D, :].reshape(VC // 128, 128, E)
        in_maps[c]["wp"] = np.ascontiguousarray(wp.astype(np.float16))

    nc = build_program()
    try:
        res = run_bass_kernel_spmd(
            nc, in_maps, core_ids=list(range(8)), trace=trace,
            trace_cores=trace_cores,
        )
    except Exception:
        # transient device wedge (e.g. NRT_EXEC_UNIT_UNRECOVERABLE) —
        # one retry is usually enough
        res = run_bass_kernel_spmd(
            nc, in_maps, core_ids=list(range(8)), trace=trace,
            trace_cores=trace_cores,
        )
    parts = [res.results[c]["y"] for c in range(8)]

    # host-folded bias: v-bias passes through softmax (weights sum to 1),
    # so y += bv @ Wproj + bproj, applied once per batch row.
    bv = bqkv_np[2 * E : 3 * E]
    bias_row = bv @ Wproj + bproj
    y = np.stack(
        [parts[2 * b] + parts[2 * b + 1] + bias_row for b in range(B)]
    ).astype(np.float32)
    return y, res


def kernel(x, mask, Wqkv, bqkv, Wproj, bproj):
    y, _ = run(x, mask, Wqkv, bqkv, Wproj, bproj, trace=False)
    return y


# revision 41
# speedup vs baseline: 1.1752x; 1.0034x over previous
"""Trainium2 Bass kernel for CodeAttention (B=4, S=2048, E=768, H=12).

Sharding: 8 cores = 4 batches x 2 head-groups (6 heads each).
Each core computes a partial projection output for its batch; the host
sums the two partials per batch and adds the (host-folded) bias row.

v3: fp16 datapath, single fused pipeline.
- The padding mask is folded multiplicatively into the V store (masked
  keys get v=0 AND ones-column=0), so exp needs no per-key-chunk bias.
- The attention kc loop is software-pipelined: scores(i) are emitted
  before pv(i-1) so the scalar engine's exp stream never waits on PE.
- QKV projection / output projection work is emitted as "filler units"
  inside the ACT-paced attention windows, keeping PE busy during exp.
"""

import sys

if "/opt/trn_rl_repo" not in sys.path:
    sys.path.insert(0, "/opt/trn_rl_repo")

import numpy as np

import concourse.bass as bass  # noqa: F401  (engine types referenced via nc)
import concourse.mybir as mybir
import concourse.tile as tile
from concourse import bacc
from concourse.alu_op_type import AluOpType
from concourse.bass_utils import run_bass_kernel_spmd
from concourse.masks import make_identity

F32 = mybir.dt.float32
F32R = mybir.dt.float32r
FP16 = mybir.dt.float16
Act = mybir.ActivationFunctionType

B, S, E, H, D = 4, 2048, 768, 12, 64
HC = 6                    # heads per core
QKC = HC * D * 2          # qk columns per core = 768
VC = HC * D               # v columns per core = 384
KCH = E // 128            # contraction chunks over E = 6
NKC = S // 128            # key chunks = 16
NQB = S // 512            # q blocks of 512 = 4
NSB = S // 512            # s blocks of 512 = 4
VW = D + 1                # v width incl. mask column = 65
NIT = NQB * (HC // 2) * NKC  # flattened attention iterations = 192


def build_program():
    nc = bacc.Bacc("TRN2", target_bir_lowering=False, debug=False, num_devices=8)

    x_d = nc.dram_tensor("x", [S, E], FP16, kind="ExternalInput")
    wqk_d = nc.dram_tensor("wqk", [QKC // 128, KCH, 128, 128], FP16, kind="ExternalInput")
    wv_d = nc.dram_tensor("wv", [KCH, 128, VC], FP16, kind="ExternalInput")
    wp_d = nc.dram_tensor("wp", [VC // 128, 128, E], FP16, kind="ExternalInput")
    bqk_d = nc.dram_tensor("bqk", [QKC], F32, kind="ExternalInput")
    mb_d = nc.dram_tensor("mb", [S], F32, kind="ExternalInput")
    mrep_d = nc.dram_tensor("mrep", [NKC * HC * 128], FP16, kind="ExternalInput")
    y_d = nc.dram_tensor("y", [S, E], F32, kind="ExternalOutput")

    with tile.TileContext(nc) as tc:
        _emit(nc, tc, x_d, wqk_d, wv_d, wp_d, bqk_d, mb_d, mrep_d, y_d)
    nc.compile()
    return nc


def _build_schedule():
    """fillers[i] = list of units to emit inside attention iteration i.

    Units: ("v", sb, sc) v-projection for s-chunk, ("qk", sb, m) qk
    projection m-chunk, ("norm", qb, hp) softmax normalization,
    ("proj", qb, sc) output projection chunk.  Lead-2 scheduling: a unit
    lands >=2 iterations before its first consumer so DVE evacuation of
    the unit's PSUM tile is off PE's critical path.
    """
    fillers = [[] for _ in range(NIT)]

    def it(qb, hp, kc):
        return (qb * (HC // 2) + hp) * NKC + kc

    # transposes for s-chunks 4..15 (sb0 is in the prologue), lead ~6
    # iterations before their first consumer (v unit / qk unit).
    for sg in range(4, NKC):
        fillers[max(0, sg - 6)].append(("tr", sg))
    # v units: consumer pv(qb0,hp0,kc) emitted at iteration kc+1.
    # (sb0,sc0) is in the prologue; the rest land at iteration kc-1.
    for kc in range(1, NKC):
        fillers[max(0, kc - 1)].append(("v", kc // 4, kc % 4))
    # k-part qk units for hp0 (m=3): scores(qb0,hp0,kc=4j) at iteration 4j.
    for j in range(1, 4):
        fillers[4 * j - 2].append(("qk", j, 3))
    # q-parts for qb0 hp1/hp2 (m=1,2 of sb0) + k-parts m=4,5:
    # scores(qb0,hp1,kc) at iteration 16+kc needs qkT[kc//4][m=4];
    # scores(qb0,hp1,*) needs qkT[0][m=1] at iteration 16.
    fillers[13].append(("qk", 0, 1))
    fillers[14].append(("qk", 0, 4))
    for j in range(1, 4):
        fillers[16 + 4 * j - 3].append(("qk", j, 4))
    fillers[16 + 9].append(("qk", 0, 2))
    fillers[16 + 12].append(("qk", 0, 5))
    for j in range(1, 4):
        fillers[32 + 4 * j - 3].append(("qk", j, 5))
    # q-part for qb1-hp0 (needed at iteration 48)
    fillers[32 + 13].append(("qk", 1, 0))
    # q-parts for later q-blocks: qkT[qb][m] needed at it(qb,hp=m,0).
    fillers[it(1, 0, 13)].append(("qk", 1, 1))
    fillers[it(1, 1, 13)].append(("qk", 1, 2))
    fillers[it(1, 2, 5)].append(("qk", 2, 0))
    fillers[it(2, 0, 5)].append(("qk", 2, 1))
    fillers[it(2, 1, 5)].append(("qk", 2, 2))
    fillers[it(2, 2, 5)].append(("qk", 3, 0))
    fillers[it(3, 0, 5)].append(("qk", 3, 1))
    fillers[it(3, 1, 5)].append(("qk", 3, 2))
    # norms: norm(qb,hp) right after pv(qb,hp,15) (emitted at +2 with the
    # depth-2 pv pipeline) so the single pv slot frees quickly.
    for qb in range(NQB):
        for hp in range(HC // 2):
            i = it(qb, hp, NKC - 1) + 3
            if i < NIT:
                fillers[i].insert(0, ("norm", qb, hp))
    # output projection of qb spread through qb+1 (norms released by then),
    # weighted toward the otherwise filler-light hp2 windows.
    for qb in range(NQB - 1):
        for sc, (hp, kc) in enumerate(((0, 6), (1, 4), (2, 2), (2, 8))):
            fillers[it(qb + 1, hp, kc)].append(("proj", qb, sc))
    return fillers


def _emit(nc, tc, x_d, wqk_d, wv_d, wp_d, bqk_d, mb_d, mrep_d, y_d):
    ctx_pools = []

    def pool(name, bufs, space="SBUF"):
        p = tc.tile_pool(name=name, bufs=bufs, space=space)
        ctx_pools.append(p)
        return p.__enter__()

    consts = pool("consts", 1)
    store = pool("store", 1)

    ident = consts.tile([128, 128], FP16)
    make_identity(nc, ident[:])
    ones_row_f = consts.tile([1, D], F32)
    nc.vector.memset(ones_row_f[:], 1.0)
    ones_row = consts.tile([1, D], F32R)
    nc.vector.tensor_copy(ones_row[:], ones_row_f[:])

    # weights go over the SWDGE (gpsimd) queue so the x-chunk loads on the
    # sync HWDGE queue aren't serialized behind the weight traffic.
    # wqk arrives per m-chunk, ordered by first use (m0/m3 feed the first
    # attention iterations).
    wqk = consts.tile([128, KCH, QKC], FP16)
    wv = consts.tile([128, KCH, VC], FP16)
    wp = consts.tile([128, VC // 128, E], FP16)

    def load_wqk(m):
        nc.gpsimd.dma_start(
            wqk[:, :, m * 128 : (m + 1) * 128],
            wqk_d.ap()[m].rearrange("k p f -> p k f"),
        )

    load_wqk(0)
    load_wqk(3)
    nc.gpsimd.dma_start(wv[:], wv_d.ap().rearrange("k p f -> p k f"))
    load_wqk(1)
    load_wqk(4)
    load_wqk(2)
    load_wqk(5)
    nc.gpsimd.dma_start(wp[:], wp_d.ap().rearrange("t p f -> p t f"))

    bqk = consts.tile([128, QKC // 128], F32)
    mb = consts.tile([128, NKC], F32)
    mrep = consts.tile([128, NKC, HC], FP16)

    # qkT store: tile m of 6 holds W-columns m*128..; q cols 0..383 (m 0..2),
    # k cols 384..767 (m 3..5).
    qkT = [
        store.tile([128, QKC // 128, 512], FP16, name=f"qkT{sb}")
        for sb in range(NSB)
    ]
    # v store: per s-block [s-chunk, head, 65] with the mask value (0/1) in
    # column 64 — multiplicative padding mask (masked keys contribute 0 to
    # both the numerator and the softmax denominator).
    vst = [
        store.tile([128, 4, HC, VW], FP16, name=f"vst{sb}") for sb in range(NSB)
    ]
    # attn output (transposed): tile t rows = head dims 2t,2t+1.
    att = store.tile([128, VC // 128, S], FP16)
    # x transposed, kept for the whole run (deferred qk/v projections).
    xts = [store.tile([128, KCH, 512], FP16, name=f"xt{sb}") for sb in range(NSB)]

    # ---- Fused pipeline pools ----
    # PSUM: st ring 3x2 banks (scores + transient unit/transpose tiles) +
    # one 2-bank pv accumulator = exactly 8 banks.
    xs_p = pool("xs", NKC)
    st_p = pool("st", 3, space="PSUM")   # [128,1024] f32 = 2 banks each
    pv_p = pool("pv", 1, space="PSUM")   # [128,1024] f32 = 2 banks
    pt_p = pool("pt", 4)
    pf_p = pool("pf", 2)
    rs_p = pool("rs", 2)
    bc_p = pool("bc", 2)
    ys_p = pool("ys", 2)

    # stage all 16 x-chunk loads up front (one SBUF tile each). The small
    # per-partition tables (contiguous per-partition runs, pre-transposed on
    # the host) slot in after the first few chunks: late enough not to delay
    # the first transposes, early enough for the first qk/v evacuations.
    xss = []
    for sg in range(NKC):
        xs = xs_p.tile([128, E], FP16, tag="xs", name=f"xs{sg}")
        nc.sync.dma_start(xs[:], x_d.ap()[sg * 128 : (sg + 1) * 128, :])
        xss.append(xs)
        if sg == 5:
            nc.scalar.dma_start(
                bqk[:], bqk_d.ap().rearrange("(p c) -> p c", p=128)
            )
            nc.scalar.dma_start(
                mb[:], mb_d.ap().rearrange("(p c) -> p c", p=128)
            )
            # mask replicated per head: mrep[p, kc, h] = mask[kc*128 + p]
            nc.scalar.dma_start(
                mrep[:], mrep_d.ap().rearrange("(p c h) -> p c h", p=128, h=HC)
            )

    def unit_tr(sg):
        sb, sc = sg // 4, sg % 4
        tp = st_p.tile([128, 1024], FP16, tag="st", name=f"tp{sg}")
        for k in range(KCH):
            nc.tensor.matmul(
                tp[:, k * 128 : (k + 1) * 128],
                xss[sg][:, k * 128 : (k + 1) * 128], ident[:],
                is_transpose=True,
                start=(k == 0), stop=(k == KCH - 1),
            )
        nc.vector.tensor_copy(
            xts[sb][:, :, sc * 128 : (sc + 1) * 128],
            tp[:, : KCH * 128].rearrange("p (k f) -> p k f", k=KCH),
        )

    def unit_v(sb, sc):
        sg = sb * 4 + sc
        u = st_p.tile([128, 1024], F32, tag="st", name=f"va{sg}")
        for k in range(KCH):
            nc.tensor.matmul(
                u[:, 0:VC], xts[sb][:, k, sc * 128 : (sc + 1) * 128],
                wv[:, k, :],
                start=(k == 0), stop=(k == KCH - 1),
            )
        # multiplicative padding mask folded into the V store (the mask
        # value for key row p is a per-partition scalar here).
        nc.vector.tensor_scalar_mul(
            vst[sb][:, sc, :, 0:D],
            u[:, 0:VC].rearrange("p (h d) -> p h d", h=HC),
            mb[:, sg : sg + 1],
        )
        nc.vector.tensor_copy(
            vst[sb][:, sc, :, D : D + 1],
            mrep[:, sg : sg + 1, :].rearrange("p one b -> p b one"),
        )

    def unit_qk(sb, m):
        u = st_p.tile([128, 1024], F32, tag="st", name=f"qk{sb}_{m}")
        for k in range(KCH):
            nc.tensor.matmul(
                u[:, 0:512], wqk[:, k, m * 128 : (m + 1) * 128], xts[sb][:, k, :],
                start=(k == 0), stop=(k == KCH - 1),
            )
        nc.vector.tensor_scalar_add(qkT[sb][:, m, :], u[:, 0:512], bqk[:, m : m + 1])

    def unit_norm(qb, hp, pvs2):
        qs = slice(qb * 512, (qb + 1) * 512)
        # evacuate the pv accumulator to SBUF fp16 right away so the single
        # 2-bank PSUM slot frees for the next head-pair (values are a few
        # thousand at most — far inside fp16 range).
        pvf = pf_p.tile([128, 1024], FP16, tag="pf", name="pvf")
        # two half-copies: each PSUM bank of the accumulator frees as soon
        # as its half is evacuated, so the next head-pair's first pv matmul
        # (same bank) unblocks ~0.5us earlier.
        for sub in range(2):
            nc.vector.tensor_copy(
                pvf[0:VW, sub * 512 : (sub + 1) * 512],
                pvs2[0:VW, sub * 512 : (sub + 1) * 512],
            )
        # reciprocal of the softmax denominator row, then rank-1 broadcast
        # to 64 partitions on the (otherwise idle) gpsimd engine.
        rse = rs_p.tile([1, 1024], F32R, tag="rs", name="rse")
        with nc.allow_low_precision(reason="f32r is full width"):
            nc.vector.reciprocal(rse[:], pvf[D : D + 1, :])
        bct = bc_p.tile([D, 1024], F32R, tag="bc", name="bct")
        nc.gpsimd.partition_broadcast(bct[:], rse[:])
        for sub in range(2):
            nc.vector.tensor_tensor(
                att[sub * 64 : sub * 64 + 64, hp, qs],
                pvf[0:D, sub * 512 : (sub + 1) * 512],
                bct[:, sub * 512 : (sub + 1) * 512],
                op=AluOpType.mult,
            )

    def unit_proj(qb, sc):
        sg = qb * 4 + sc
        ys = ys_p.tile([128, E], F32, tag="ys")
        for n0, nw in ((0, 512), (512, 256)):
            ya = st_p.tile([128, 1024], F32, tag="st", name="ya")
            for t in range(VC // 128):
                nc.tensor.matmul(
                    ya[:, :nw],
                    att[:, t, sg * 128 : (sg + 1) * 128],
                    wp[:, t, n0 : n0 + nw],
                    start=(t == 0), stop=(t == VC // 128 - 1),
                )
            nc.vector.tensor_copy(ys[:, n0 : n0 + nw], ya[:, :nw])
            # per-half store overlaps the DMA with the second half's matmuls
            nc.sync.dma_start(
                y_d.ap()[sg * 128 : (sg + 1) * 128, n0 : n0 + nw],
                ys[:, n0 : n0 + nw],
            )

    def emit_unit(u, state):
        kind = u[0]
        if kind == "tr":
            unit_tr(u[1])
        elif kind == "v":
            unit_v(u[1], u[2])
        elif kind == "qk":
            unit_qk(u[1], u[2])
        elif kind == "norm":
            unit_norm(u[1], u[2], state["pvs2"].pop((u[1], u[2])))
        elif kind == "proj":
            unit_proj(u[1], u[2])

    # prologue units: enough to start (qb0, hp0, kc0).
    for sg in range(4):
        unit_tr(sg)
    unit_qk(0, 0)
    unit_qk(0, 3)
    unit_v(0, 0)

    fillers = _build_schedule()
    state = {"pvs2": {}}
    pending = []  # (pt tile, qb, hp, kc) awaiting pv matmuls (depth-2 pipe)

    def emit_pv(prev):
        pt, qb, hp, kc = prev
        pvs2 = state["pvs2"][(qb, hp)]
        for sub in range(2):
            h = hp * 2 + sub
            nc.tensor.matmul(
                pvs2[0:VW, sub * 512 : (sub + 1) * 512],
                vst[kc // 4][:, kc % 4, h, :],
                pt[:, sub * 512 : (sub + 1) * 512],
                start=(kc == 0), stop=(kc == NKC - 1),
            )

    for i in range(NIT):
        kc = i % NKC
        hp = (i // NKC) % (HC // 2)
        qb = i // (NKC * (HC // 2))
        if kc == 0:
            state["pvs2"][(qb, hp)] = pv_p.tile(
                [128, 1024], F32, tag="pv", name=f"pv{qb}_{hp}"
            )
        # scores: both heads of the pair into one 2-bank f32 tile so one
        # exp instruction (free dim 1024) covers both.
        st = st_p.tile([128, 1024], F32, tag="st")
        for sub in range(2):
            kb, ko = kc // 4, kc % 4
            r0 = sub * 64
            nc.tensor.matmul(
                st[:, sub * 512 : (sub + 1) * 512],
                qkT[kb][r0 : r0 + 64, 3 + hp, ko * 128 : (ko + 1) * 128],
                qkT[qb][r0 : r0 + 64, hp, :],
                start=True, stop=True,
            )
        pt = pt_p.tile([128, 1024], FP16, tag="pt")
        nc.scalar.activation(pt[:], st[:], Act.Exp, scale=0.125)
        for u in fillers[i]:
            emit_unit(u, state)
        if len(pending) >= 2:
            emit_pv(pending.pop(0))
        pending.append((pt, qb, hp, kc))
    for p in pending:
        emit_pv(p)

    # tail: last head-pair's norm with the shortest possible chain — read
    # the pv accumulator in place and broadcast on PE (idle by now).
    pvs2 = state["pvs2"].pop((NQB - 1, HC // 2 - 1))
    qs = slice((NQB - 1) * 512, NQB * 512)
    rse = rs_p.tile([1, 1024], F32R, tag="rs", name="rse_t")
    with nc.allow_low_precision(reason="f32r is full width"):
        nc.vector.reciprocal(rse[:], pvs2[D : D + 1, :])
    pvf = pf_p.tile([128, 1024], FP16, tag="pf", name="pvf_t")
    nc.vector.tensor_copy(pvf[0:D, :], pvs2[0:D, :])
    # start the last q-block's projection on the heads that are already
    # normalized (t=0,1) while the reciprocal/broadcast chain drains; the
    # t=2 accumulation step joins after the final normalize below.
    yas, yss = [], []
    for sc in range(2):
        sg = (NQB - 1) * 4 + sc
        ya = st_p.tile([128, 1024], F32, tag="st", name=f"yat{sc}")
        for n0, nw in ((0, 512), (512, 256)):
            for t in range(2):
                nc.tensor.matmul(
                    ya[:, n0 : n0 + nw] if n0 == 0 else ya[:, 512 : 512 + nw],
                    att[:, t, sg * 128 : (sg + 1) * 128],
                    wp[:, t, n0 : n0 + nw],
                    start=(t == 0), stop=False,
                )
        yas.append(ya)
    bcp = st_p.tile([128, 1024], F32, tag="st", name="bcp")
    for sub in range(2):
        nc.tensor.matmul(
            bcp[0:D, sub * 512 : (sub + 1) * 512],
            ones_row[:], rse[:, sub * 512 : (sub + 1) * 512],
            start=True, stop=True,
        )
    for sub in range(2):
        nc.vector.tensor_tensor(
            att[sub * 64 : sub * 64 + 64, HC // 2 - 1, qs],
            pvf[0:D, sub * 512 : (sub + 1) * 512],
            bcp[0:D, sub * 512 : (sub + 1) * 512],
            op=AluOpType.mult,
        )
    for sc in range(2):
        sg = (NQB - 1) * 4 + sc
        ya = yas[sc]
        ys = ys_p.tile([128, E], F32, tag="ys")
        for n0, nw in ((0, 512), (512, 256)):
            nc.tensor.matmul(
                ya[:, n0 : n0 + nw],
                att[:, 2, sg * 128 : (sg + 1) * 128],
                wp[:, 2, n0 : n0 + nw],
                start=False, stop=True,
            )
            nc.vector.tensor_copy(ys[:, n0 : n0 + nw], ya[:, n0 : n0 + nw])
            nc.sync.dma_start(
                y_d.ap()[sg * 128 : (sg + 1) * 128, n0 : n0 + nw],
                ys[:, n0 : n0 + nw],
            )
    for sc in range(2, 4):
        unit_proj(NQB - 1, sc)

    for p in reversed(ctx_pools):
        p.__exit__(None, None, None)


def make_core_inputs(x, mask, Wqkv, bqkv):
    """Slice full inputs into 8 per-core input maps."""
    x = np.asarray(x, dtype=np.float32)
    mask = np.asarray(mask)
    Wqkv = np.asarray(Wqkv, dtype=np.float32)
    bqkv = np.asarray(bqkv, dtype=np.float32)
    in_maps = []
    for c in range(8):
        b = c // 2
        h0 = (c % 2) * HC
        wq = Wqkv[:, h0 * D : (h0 + HC) * D]
        wk = Wqkv[:, E + h0 * D : E + (h0 + HC) * D]
        # [m, k, 128, 128]: per-m-chunk DMA granularity
        wqk = np.concatenate([wq, wk], axis=1).reshape(KCH, 128, QKC // 128, 128)
        wqk = wqk.transpose(2, 0, 1, 3)
        wv = Wqkv[:, 2 * E + h0 * D : 2 * E + (h0 + HC) * D].reshape(KCH, 128, VC)
        bqk = np.concatenate(
            [bqkv[h0 * D : (h0 + HC) * D], bqkv[E + h0 * D : E + (h0 + HC) * D]]
        )
        mv = (mask[b, 0, 0, :] != 0).astype(np.float32)  # 1.0 keep, 0.0 drop
        mb_t = mv.reshape(NKC, 128).T  # [p, c]
        mrep = np.repeat(mb_t[:, :, None], HC, axis=2)  # [p, c, h]
        in_maps.append(
            {
                "x": np.ascontiguousarray(x[b].astype(np.float16)),
                "wqk": np.ascontiguousarray(wqk.astype(np.float16)),
                "wv": np.ascontiguousarray(wv.astype(np.float16)),
                "wp": None,  # filled below (needs Wproj)
                "bqk": np.ascontiguousarray(
                    bqk.reshape(QKC // 128, 128).T.astype(np.float32).ravel()
                ),
                "mb": np.ascontiguousarray(mb_t.astype(np.float32).ravel()),
                "mrep": np.ascontiguousarray(mrep.astype(np.float16).ravel()),
            }
        )
    return in_maps


def run(x, mask, Wqkv, bqkv, Wproj, bproj, trace=False, trace_cores=None):
    Wproj = np.asarray(Wproj, dtype=np.float32)
    bproj = np.asarray(bproj, dtype=np.float32)
    bqkv_np = np.asarray(bqkv, dtype=np.float32)
    in_maps = make_core_inputs(x, mask, Wqkv, bqkv_np)
    for c in range(8):
        h0 = (c % 2) * HC
        wp = Wproj[h0 * D : (h0 + HC) * D, :].reshape(VC // 128, 128, E)
        in_maps[c]["wp"] = np.ascontiguousarray(wp.astype(np.float16))

    nc = build_program()
    try:
        res = run_bass_kernel_spmd(
            nc, in_maps, core_ids=list(range(8)), trace=trace,
            trace_cores=trace_cores,
        )
    except Exception:
        # transient device wedge (e.g. NRT_EXEC_UNIT_UNRECOVERABLE) —
        # one retry is usually enough
        res = run_bass_kernel_spmd(
            nc, in_maps, core_ids=list(range(8)), trace=trace,
            trace_cores=trace_cores,
        )
    parts = [res.results[c]["y"] for c in range(8)]

    # host-folded bias: v-bias passes through softmax (weights sum to 1),
    # so y += bv @ Wproj + bproj, applied once per batch row.
    bv = bqkv_np[2 * E : 3 * E]
    bias_row = bv @ Wproj + bproj
    y = np.stack(
        [parts[2 * b] + parts[2 * b + 1] + bias_row for b in range(B)]
    ).astype(np.float32)
    return y, res


def kernel(x, mask, Wqkv, bqkv, Wproj, bproj):
    y, _ = run(x, mask, Wqkv, bqkv, Wproj, bproj, trace=False)
    return y


# revision 42
# speedup vs baseline: 1.1794x; 1.0036x over previous
"""Trainium2 Bass kernel for CodeAttention (B=4, S=2048, E=768, H=12).

Sharding: 8 cores = 4 batches x 2 head-groups (6 heads each).
Each core computes a partial projection output for its batch; the host
sums the two partials per batch and adds the (host-folded) bias row.

v3: fp16 datapath, single fused pipeline.
- The padding mask is folded multiplicatively into the V store (masked
  keys get v=0 AND ones-column=0), so exp needs no per-key-chunk bias.
- The attention kc loop is software-pipelined: scores(i) are emitted
  before pv(i-1) so the scalar engine's exp stream never waits on PE.
- QKV projection / output projection work is emitted as "filler units"
  inside the ACT-paced attention windows, keeping PE busy during exp.
"""

import sys

if "/opt/trn_rl_repo" not in sys.path:
    sys.path.insert(0, "/opt/trn_rl_repo")

import numpy as np

import concourse.bass as bass  # noqa: F401  (engine types referenced via nc)
import concourse.mybir as mybir
import concourse.tile as tile
from concourse import bacc
from concourse.alu_op_type import AluOpType
from concourse.bass_utils import run_bass_kernel_spmd
from concourse.masks import make_identity

F32 = mybir.dt.float32
F32R = mybir.dt.float32r
FP16 = mybir.dt.float16
Act = mybir.ActivationFunctionType

B, S, E, H, D = 4, 2048, 768, 12, 64
HC = 6                    # heads per core
QKC = HC * D * 2          # qk columns per core = 768
VC = HC * D               # v columns per core = 384
KCH = E // 128            # contraction chunks over E = 6
NKC = S // 128            # key chunks = 16
NQB = S // 512            # q blocks of 512 = 4
NSB = S // 512            # s blocks of 512 = 4
VW = D + 1                # v width incl. mask column = 65
NIT = NQB * (HC // 2) * NKC  # flattened attention iterations = 192


def build_program():
    nc = bacc.Bacc("TRN2", target_bir_lowering=False, debug=False, num_devices=8)

    x_d = nc.dram_tensor("x", [S, E], FP16, kind="ExternalInput")
    wqk_d = nc.dram_tensor("wqk", [QKC // 128, KCH, 128, 128], FP16, kind="ExternalInput")
    wv_d = nc.dram_tensor("wv", [KCH, 128, VC], FP16, kind="ExternalInput")
    wp_d = nc.dram_tensor("wp", [VC // 128, 128, E], FP16, kind="ExternalInput")
    bqk_d = nc.dram_tensor("bqk", [QKC], F32, kind="ExternalInput")
    mb_d = nc.dram_tensor("mb", [S], F32, kind="ExternalInput")
    mrep_d = nc.dram_tensor("mrep", [NKC * HC * 128], FP16, kind="ExternalInput")
    y_d = nc.dram_tensor("y", [S, E], F32, kind="ExternalOutput")

    with tile.TileContext(nc) as tc:
        _emit(nc, tc, x_d, wqk_d, wv_d, wp_d, bqk_d, mb_d, mrep_d, y_d)
    nc.compile()
    return nc


def _build_schedule():
    """fillers[i] = list of units to emit inside attention iteration i.

    Units: ("v", sb, sc) v-projection for s-chunk, ("qk", sb, m) qk
    projection m-chunk, ("norm", qb, hp) softmax normalization,
    ("proj", qb, sc) output projection chunk.  Lead-2 scheduling: a unit
    lands >=2 iterations before its first consumer so DVE evacuation of
    the unit's PSUM tile is off PE's critical path.
    """
    fillers = [[] for _ in range(NIT)]

    def it(qb, hp, kc):
        return (qb * (HC // 2) + hp) * NKC + kc

    # transposes for s-chunks 4..15 (sb0 is in the prologue), lead ~6
    # iterations before their first consumer (v unit / qk unit).
    for sg in range(4, NKC):
        fillers[max(0, sg - 6)].append(("tr", sg))
    # v units: consumer pv(qb0,hp0,kc) emitted at iteration kc+1.
    # (sb0,sc0) is in the prologue; the rest land at iteration kc-1.
    for kc in range(1, NKC):
        fillers[max(0, kc - 1)].append(("v", kc // 4, kc % 4))
    # k-part qk units for hp0 (m=3): scores(qb0,hp0,kc=4j) at iteration 4j.
    for j in range(1, 4):
        fillers[4 * j - 2].append(("qk", j, 3))
    # q-parts for qb0 hp1/hp2 (m=1,2 of sb0) + k-parts m=4,5:
    # scores(qb0,hp1,kc) at iteration 16+kc needs qkT[kc//4][m=4];
    # scores(qb0,hp1,*) needs qkT[0][m=1] at iteration 16.
    fillers[13].append(("qk", 0, 1))
    fillers[14].append(("qk", 0, 4))
    for j in range(1, 4):
        fillers[16 + 4 * j - 3].append(("qk", j, 4))
    fillers[16 + 9].append(("qk", 0, 2))
    fillers[16 + 12].append(("qk", 0, 5))
    for j in range(1, 4):
        fillers[32 + 4 * j - 3].append(("qk", j, 5))
    # q-part for qb1-hp0 (needed at iteration 48)
    fillers[32 + 13].append(("qk", 1, 0))
    # q-parts for later q-blocks: qkT[qb][m] needed at it(qb,hp=m,0).
    fillers[it(1, 0, 13)].append(("qk", 1, 1))
    fillers[it(1, 1, 13)].append(("qk", 1, 2))
    fillers[it(1, 2, 5)].append(("qk", 2, 0))
    fillers[it(2, 0, 5)].append(("qk", 2, 1))
    fillers[it(2, 1, 5)].append(("qk", 2, 2))
    fillers[it(2, 2, 5)].append(("qk", 3, 0))
    fillers[it(3, 0, 5)].append(("qk", 3, 1))
    fillers[it(3, 1, 5)].append(("qk", 3, 2))
    # norms: norm(qb,hp) right after pv(qb,hp,15) (emitted at +2 with the
    # depth-2 pv pipeline) so the single pv slot frees quickly.
    for qb in range(NQB):
        for hp in range(HC // 2):
            i = it(qb, hp, NKC - 1) + 3
            if i < NIT:
                fillers[i].insert(0, ("norm", qb, hp))
    # output projection of qb spread through qb+1 (norms released by then),
    # weighted toward the otherwise filler-light hp2 windows.
    for qb in range(NQB - 1):
        for sc, (hp, kc) in enumerate(((0, 6), (1, 4), (2, 2), (2, 8))):
            fillers[it(qb + 1, hp, kc)].append(("proj", qb, sc))
    return fillers


def _emit(nc, tc, x_d, wqk_d, wv_d, wp_d, bqk_d, mb_d, mrep_d, y_d):
    ctx_pools = []

    def pool(name, bufs, space="SBUF"):
        p = tc.tile_pool(name=name, bufs=bufs, space=space)
        ctx_pools.append(p)
        return p.__enter__()

    consts = pool("consts", 1)
    store = pool("store", 1)

    ident = consts.tile([128, 128], FP16)
    make_identity(nc, ident[:])
    ones_row_f = consts.tile([1, D], F32)
    nc.vector.memset(ones_row_f[:], 1.0)
    ones_row = consts.tile([1, D], F32R)
    nc.vector.tensor_copy(ones_row[:], ones_row_f[:])

    # weights go over the SWDGE (gpsimd) queue so the x-chunk loads on the
    # sync HWDGE queue aren't serialized behind the weight traffic.
    # wqk arrives per m-chunk, ordered by first use (m0/m3 feed the first
    # attention iterations).
    wqk = consts.tile([128, KCH, QKC], FP16)
    wv = consts.tile([128, KCH, VC], FP16)
    wp = consts.tile([128, VC // 128, E], FP16)

    def load_wqk(m):
        nc.gpsimd.dma_start(
            wqk[:, :, m * 128 : (m + 1) * 128],
            wqk_d.ap()[m].rearrange("k p f -> p k f"),
        )

    load_wqk(0)
    load_wqk(3)
    nc.gpsimd.dma_start(wv[:], wv_d.ap().rearrange("k p f -> p k f"))
    load_wqk(1)
    load_wqk(4)
    load_wqk(2)
    load_wqk(5)
    nc.gpsimd.dma_start(wp[:], wp_d.ap().rearrange("t p f -> p t f"))

    bqk = consts.tile([128, QKC // 128], F32)
    mb = consts.tile([128, NKC], F32)
    mrep = consts.tile([128, NKC, HC], FP16)

    # qkT store: tile m of 6 holds W-columns m*128..; q cols 0..383 (m 0..2),
    # k cols 384..767 (m 3..5).
    qkT = [
        store.tile([128, QKC // 128, 512], FP16, name=f"qkT{sb}")
        for sb in range(NSB)
    ]
    # v store: per s-block [s-chunk, head, 65] with the mask value (0/1) in
    # column 64 — multiplicative padding mask (masked keys contribute 0 to
    # both the numerator and the softmax denominator).
    vst = [
        store.tile([128, 4, HC, VW], FP16, name=f"vst{sb}") for sb in range(NSB)
    ]
    # attn output (transposed): tile t rows = head dims 2t,2t+1.
    att = store.tile([128, VC // 128, S], FP16)
    # x transposed, kept for the whole run (deferred qk/v projections).
    xts = [store.tile([128, KCH, 512], FP16, name=f"xt{sb}") for sb in range(NSB)]

    # ---- Fused pipeline pools ----
    # PSUM: st ring 3x2 banks (scores + transient unit/transpose tiles) +
    # one 2-bank pv accumulator = exactly 8 banks.
    xs_p = pool("xs", NKC)
    st_p = pool("st", 3, space="PSUM")   # [128,1024] f32 = 2 banks each
    pv_p = pool("pv", 1, space="PSUM")   # [128,1024] f32 = 2 banks
    pt_p = pool("pt", 4)
    pf_p = pool("pf", 2)
    rs_p = pool("rs", 2)
    bc_p = pool("bc", 2)
    ys_p = pool("ys", 2)

    # stage all 16 x-chunk loads up front (one SBUF tile each). The small
    # per-partition tables (contiguous per-partition runs, pre-transposed on
    # the host) slot in after the first few chunks: late enough not to delay
    # the first transposes, early enough for the first qk/v evacuations.
    xss = []
    for sg in range(NKC):
        xs = xs_p.tile([128, E], FP16, tag="xs", name=f"xs{sg}")
        nc.sync.dma_start(xs[:], x_d.ap()[sg * 128 : (sg + 1) * 128, :])
        xss.append(xs)
        if sg == 5:
            nc.scalar.dma_start(
                bqk[:], bqk_d.ap().rearrange("(p c) -> p c", p=128)
            )
            nc.scalar.dma_start(
                mb[:], mb_d.ap().rearrange("(p c) -> p c", p=128)
            )
            # mask replicated per head: mrep[p, kc, h] = mask[kc*128 + p]
            nc.scalar.dma_start(
                mrep[:], mrep_d.ap().rearrange("(p c h) -> p c h", p=128, h=HC)
            )

    def unit_tr(sg):
        sb, sc = sg // 4, sg % 4
        tp = st_p.tile([128, 1024], FP16, tag="st", name=f"tp{sg}")
        for k in range(KCH):
            nc.tensor.matmul(
                tp[:, k * 128 : (k + 1) * 128],
                xss[sg][:, k * 128 : (k + 1) * 128], ident[:],
                is_transpose=True,
                start=(k == 0), stop=(k == KCH - 1),
            )
        nc.vector.tensor_copy(
            xts[sb][:, :, sc * 128 : (sc + 1) * 128],
            tp[:, : KCH * 128].rearrange("p (k f) -> p k f", k=KCH),
        )

    def unit_v(sb, sc):
        sg = sb * 4 + sc
        u = st_p.tile([128, 1024], F32, tag="st", name=f"va{sg}")
        for k in range(KCH):
            nc.tensor.matmul(
                u[:, 0:VC], xts[sb][:, k, sc * 128 : (sc + 1) * 128],
                wv[:, k, :],
                start=(k == 0), stop=(k == KCH - 1),
            )
        # multiplicative padding mask folded into the V store (the mask
        # value for key row p is a per-partition scalar here).
        nc.vector.tensor_scalar_mul(
            vst[sb][:, sc, :, 0:D],
            u[:, 0:VC].rearrange("p (h d) -> p h d", h=HC),
            mb[:, sg : sg + 1],
        )
        nc.vector.tensor_copy(
            vst[sb][:, sc, :, D : D + 1],
            mrep[:, sg : sg + 1, :].rearrange("p one b -> p b one"),
        )

    def unit_qk(sb, m):
        u = st_p.tile([128, 1024], F32, tag="st", name=f"qk{sb}_{m}")
        for k in range(KCH):
            nc.tensor.matmul(
                u[:, 0:512], wqk[:, k, m * 128 : (m + 1) * 128], xts[sb][:, k, :],
                start=(k == 0), stop=(k == KCH - 1),
            )
        nc.vector.tensor_scalar_add(qkT[sb][:, m, :], u[:, 0:512], bqk[:, m : m + 1])

    def unit_norm(qb, hp, pvs2):
        qs = slice(qb * 512, (qb + 1) * 512)
        # evacuate the pv accumulator to SBUF fp16 right away so the single
        # 2-bank PSUM slot frees for the next head-pair (values are a few
        # thousand at most — far inside fp16 range).
        pvf = pf_p.tile([128, 1024], FP16, tag="pf", name="pvf")
        # two half-copies: each PSUM bank of the accumulator frees as soon
        # as its half is evacuated, so the next head-pair's first pv matmul
        # (same bank) unblocks ~0.5us earlier.
        for sub in range(2):
            nc.vector.tensor_copy(
                pvf[0:VW, sub * 512 : (sub + 1) * 512],
                pvs2[0:VW, sub * 512 : (sub + 1) * 512],
            )
        # reciprocal of the softmax denominator row, then rank-1 broadcast
        # to 64 partitions on the (otherwise idle) gpsimd engine.
        rse = rs_p.tile([1, 1024], F32R, tag="rs", name="rse")
        with nc.allow_low_precision(reason="f32r is full width"):
            nc.vector.reciprocal(rse[:], pvf[D : D + 1, :])
        bct = bc_p.tile([D, 1024], F32R, tag="bc", name="bct")
        nc.gpsimd.partition_broadcast(bct[:], rse[:])
        for sub in range(2):
            nc.vector.tensor_tensor(
                att[sub * 64 : sub * 64 + 64, hp, qs],
                pvf[0:D, sub * 512 : (sub + 1) * 512],
                bct[:, sub * 512 : (sub + 1) * 512],
                op=AluOpType.mult,
            )

    def unit_proj(qb, sc):
        sg = qb * 4 + sc
        ys = ys_p.tile([128, E], F32, tag="ys")
        for n0, nw in ((0, 512), (512, 256)):
            ya = st_p.tile([128, 1024], F32, tag="st", name="ya")
            for t in range(VC // 128):
                nc.tensor.matmul(
                    ya[:, :nw],
                    att[:, t, sg * 128 : (sg + 1) * 128],
                    wp[:, t, n0 : n0 + nw],
                    start=(t == 0), stop=(t == VC // 128 - 1),
                )
            nc.vector.tensor_copy(ys[:, n0 : n0 + nw], ya[:, :nw])
            # per-half store overlaps the DMA with the second half's matmuls
            nc.sync.dma_start(
                y_d.ap()[sg * 128 : (sg + 1) * 128, n0 : n0 + nw],
                ys[:, n0 : n0 + nw],
            )

    def emit_unit(u, state):
        kind = u[0]
        if kind == "tr":
            unit_tr(u[1])
        elif kind == "v":
            unit_v(u[1], u[2])
        elif kind == "qk":
            unit_qk(u[1], u[2])
        elif kind == "norm":
            unit_norm(u[1], u[2], state["pvs2"].pop((u[1], u[2])))
        elif kind == "proj":
            unit_proj(u[1], u[2])

    # prologue units: enough to start (qb0, hp0, kc0).
    for sg in range(4):
        unit_tr(sg)
    unit_qk(0, 0)
    unit_qk(0, 3)
    unit_v(0, 0)

    fillers = _build_schedule()
    state = {"pvs2": {}}
    pending = []  # (pt tile, qb, hp, kc) awaiting pv matmuls (depth-2 pipe)

    def emit_pv(prev):
        pt, qb, hp, kc = prev
        pvs2 = state["pvs2"][(qb, hp)]
        for sub in range(2):
            h = hp * 2 + sub
            nc.tensor.matmul(
                pvs2[0:VW, sub * 512 : (sub + 1) * 512],
                vst[kc // 4][:, kc % 4, h, :],
                pt[:, sub * 512 : (sub + 1) * 512],
                start=(kc == 0), stop=(kc == NKC - 1),
            )

    for i in range(NIT):
        kc = i % NKC
        hp = (i // NKC) % (HC // 2)
        qb = i // (NKC * (HC // 2))
        if kc == 0:
            state["pvs2"][(qb, hp)] = pv_p.tile(
                [128, 1024], F32, tag="pv", name=f"pv{qb}_{hp}"
            )
        # scores: both heads of the pair into one 2-bank f32 tile so one
        # exp instruction (free dim 1024) covers both.
        st = st_p.tile([128, 1024], F32, tag="st")
        for sub in range(2):
            kb, ko = kc // 4, kc % 4
            r0 = sub * 64
            nc.tensor.matmul(
                st[:, sub * 512 : (sub + 1) * 512],
                qkT[kb][r0 : r0 + 64, 3 + hp, ko * 128 : (ko + 1) * 128],
                qkT[qb][r0 : r0 + 64, hp, :],
                start=True, stop=True,
            )
        pt = pt_p.tile([128, 1024], FP16, tag="pt")
        nc.scalar.activation(pt[:], st[:], Act.Exp, scale=0.125)
        for u in fillers[i]:
            emit_unit(u, state)
        if len(pending) >= 2:
            emit_pv(pending.pop(0))
        pending.append((pt, qb, hp, kc))
    for p in pending:
        emit_pv(p)

    # tail: last head-pair's norm with the shortest possible chain — read
    # the pv accumulator in place and broadcast on PE (idle by now).
    pvs2 = state["pvs2"].pop((NQB - 1, HC // 2 - 1))
    qs = slice((NQB - 1) * 512, NQB * 512)
    # per-head reciprocal + gpsimd broadcast halves so each head's multiply
    # starts as soon as its own chain drains; the multiply reads the pv
    # accumulator in place (PSUM x SBUF — no evacuation copy needed).
    bct = bc_p.tile([D, 1024], F32R, tag="bc", name="bct_t")
    rse = rs_p.tile([1, 1024], F32R, tag="rs", name="rse_t")
    for sub in range(2):
        half = slice(sub * 512, (sub + 1) * 512)
        with nc.allow_low_precision(reason="f32r is full width"):
            nc.vector.reciprocal(rse[:, half], pvs2[D : D + 1, half])
        nc.gpsimd.partition_broadcast(bct[:, half], rse[:, half])
    # start the last q-block's projection on the heads that are already
    # normalized (t=0,1) while the reciprocal/broadcast chain drains; the
    # t=2 accumulation step joins after the final normalize below.
    yas = []
    for sc in range(3):
        sg = (NQB - 1) * 4 + sc
        ya = st_p.tile([128, 1024], F32, tag="st", name=f"yat{sc}")
        for n0, nw in ((0, 512), (512, 256)):
            for t in range(2):
                nc.tensor.matmul(
                    ya[:, n0 : n0 + nw],
                    att[:, t, sg * 128 : (sg + 1) * 128],
                    wp[:, t, n0 : n0 + nw],
                    start=(t == 0), stop=False,
                )
        yas.append(ya)
    for sub in range(2):
        nc.vector.tensor_tensor(
            att[sub * 64 : sub * 64 + 64, HC // 2 - 1, qs],
            pvs2[0:D, sub * 512 : (sub + 1) * 512],
            bct[:, sub * 512 : (sub + 1) * 512],
            op=AluOpType.mult,
        )
    for sc in range(3):
        sg = (NQB - 1) * 4 + sc
        ya = yas[sc]
        ys = ys_p.tile([128, E], F32, tag="ys")
        for n0, nw in ((0, 512), (512, 256)):
            nc.tensor.matmul(
                ya[:, n0 : n0 + nw],
                att[:, 2, sg * 128 : (sg + 1) * 128],
                wp[:, 2, n0 : n0 + nw],
                start=False, stop=True,
            )
            nc.vector.tensor_copy(ys[:, n0 : n0 + nw], ya[:, n0 : n0 + nw])
            nc.sync.dma_start(
                y_d.ap()[sg * 128 : (sg + 1) * 128, n0 : n0 + nw],
                ys[:, n0 : n0 + nw],
            )
    unit_proj(NQB - 1, 3)

    for p in reversed(ctx_pools):
        p.__exit__(None, None, None)


def make_core_inputs(x, mask, Wqkv, bqkv):
    """Slice full inputs into 8 per-core input maps."""
    x = np.asarray(x, dtype=np.float32)
    mask = np.asarray(mask)
    Wqkv = np.asarray(Wqkv, dtype=np.float32)
    bqkv = np.asarray(bqkv, dtype=np.float32)
    in_maps = []
    for c in range(8):
        b = c // 2
        h0 = (c % 2) * HC
        wq = Wqkv[:, h0 * D : (h0 + HC) * D]
        wk = Wqkv[:, E + h0 * D : E + (h0 + HC) * D]
        # [m, k, 128, 128]: per-m-chunk DMA granularity
        wqk = np.concatenate([wq, wk], axis=1).reshape(KCH, 128, QKC // 128, 128)
        wqk = wqk.transpose(2, 0, 1, 3)
        wv = Wqkv[:, 2 * E + h0 * D : 2 * E + (h0 + HC) * D].reshape(KCH, 128, VC)
        bqk = np.concatenate(
            [bqkv[h0 * D : (h0 + HC) * D], bqkv[E + h0 * D : E + (h0 + HC) * D]]
        )
        mv = (mask[b, 0, 0, :] != 0).astype(np.float32)  # 1.0 keep, 0.0 drop
        mb_t = mv.reshape(NKC, 128).T  # [p, c]
        mrep = np.repeat(mb_t[:, :, None], HC, axis=2)  # [p, c, h]
        in_maps.append(
            {
                "x": np.ascontiguousarray(x[b].astype(np.float16)),
                "wqk": np.ascontiguousarray(wqk.astype(np.float16)),
                "wv": np.ascontiguousarray(wv.astype(np.float16)),
                "wp": None,  # filled below (needs Wproj)
                "bqk": np.ascontiguousarray(
                    bqk.reshape(QKC // 128, 128).T.astype(np.float32).ravel()
                ),
                "mb": np.ascontiguousarray(mb_t.astype(np.float32).ravel()),
                "mrep": np.ascontiguousarray(mrep.astype(np.float16).ravel()),
            }
        )
    return in_maps


def run(x, mask, Wqkv, bqkv, Wproj, bproj, trace=False, trace_cores=None):
    Wproj = np.asarray(Wproj, dtype=np.float32)
    bproj = np.asarray(bproj, dtype=np.float32)
    bqkv_np = np.asarray(bqkv, dtype=np.float32)
    in_maps = make_core_inputs(x, mask, Wqkv, bqkv_np)
    for c in range(8):
        h0 = (c % 2) * HC
        wp = Wproj[h0 * D : (h0 + HC) * D, :].reshape(VC // 128, 128, E)
        in_maps[c]["wp"] = np.ascontiguousarray(wp.astype(np.float16))

    nc = build_program()
    try:
        res = run_bass_kernel_spmd(
            nc, in_maps, core_ids=list(range(8)), trace=trace,
            trace_cores=trace_cores,
        )
    except Exception:
        # transient device wedge (e.g. NRT_EXEC_UNIT_UNRECOVERABLE) —
        # one retry is usually enough
        res = run_bass_kernel_spmd(
            nc, in_maps, core_ids=list(range(8)), trace=trace,
            trace_cores=trace_cores,
        )
    parts = [res.results[c]["y"] for c in range(8)]

    # host-folded bias: v-bias passes through softmax (weights sum to 1),
    # so y += bv @ Wproj + bproj, applied once per batch row.
    bv = bqkv_np[2 * E : 3 * E]
    bias_row = bv @ Wproj + bproj
    y = np.stack(
        [parts[2 * b] + parts[2 * b + 1] + bias_row for b in range(B)]
    ).astype(np.float32)
    return y, res


def kernel(x, mask, Wqkv, bqkv, Wproj, bproj):
    y, _ = run(x, mask, Wqkv, bqkv, Wproj, bproj, trace=False)
    return y
